# revision 42
# baseline (speedup 1.0000x reference)
"""Trainium2 Bass kernel for nn_Attention_4045859193206 (Swin-style window
attention with relative position bias + key masking).

Contract: kernel(**inputs) takes FULL inputs (B=128 windows), shards the batch
across 8 NeuronCores (16 windows each), runs one SPMD Bass kernel, returns the
FULL (128, 196, 512) float32 output.

Self-contained: hardcodes all shapes; no sibling imports.

Design (per core, W=16 windows):
  - x cast fp32->fp16 on device (DRAM->DRAM DMA cast), then DMA-transposed to
    x^T [c, tok] in SBUF (4-window groups: 784 tokens, multiple of 16).
  - QKV: Q^T/K^T computed in transposed form ([o,tok], fp16, q pre-scaled via
    host-scaled weights); V in natural form ([tok,o]).
  - S^T = K^T-lhsT matmuls, 4 heads row-packed (tile_position), RPE bias added
    by selection-matrix matmuls accumulating into the same PSUM banks.
  - softmax: P = exp(S + mask_bias - 4) via ScalarE (per-partition bias AP);
    the -4 shift cancels in normalization and keeps fp16 in range.
  - O^T = V-lhsT matmuls col-packed over 4 heads; Z via ones-matmul col-packed
    (rows replicated 32x so the reciprocal is already partition-aligned).
  - normalize O^T with vector.reciprocal (on PSUM) + tensor_mul; proj in natural
    output layout (lhsT = O^T chunks), proj bias added during the final
    PSUM->SBUF pass, DMA out.
  - RPE bias table gathered on-device with gpsimd.dma_gather (rows padded to
    256B), into [k-part, (chunk,q), h] layout.
"""

import contextlib
import numpy as np

import concourse.bass as bass
import concourse.mybir as mybir
import concourse.tile as tile
from concourse.bacc import Bacc

# ---------------------------------------------------------------- constants
B, N, DIM, H = 128, 196, 512, 16
HD = DIM // H                     # 32
RPE = 729                         # (2*14-1)^2
NCORES = 8
W = B // NCORES                   # 16 windows per core
NKC = 98                          # k-chunk (2 chunks of 98 = 196)
GW = 4                            # windows per qkv group (4*196=784 tokens)
F16 = mybir.dt.float16
F32 = mybir.dt.float32
I16 = mybir.dt.int16
EXP_SHIFT = -4.0                  # exp(s-4): fp16 headroom; cancels in softmax
MASK_NEG = -1e9
_GQ = 98                          # (c,q) positions per gather chunk
_NGATHER = 4                      # 4 chunks of 98 positions = 392


def _build_nc(n_w=W, ablate=frozenset(), variant="base"):
    """Build the per-core Bass program for n_w windows.
    ablate: subset of {'z','bias','qk','pv'} - drop those matmuls (timing expts).
    variant: 'base' or 'bundle2' (2-head bias bundling + s_ps 2 banks x 2 bufs)."""
    assert n_w % GW == 0
    ngrp = n_w // GW
    nc = Bacc("TRN2", target_bir_lowering=False)

    x_d = nc.dram_tensor("x", (n_w, N, DIM), F32, kind="ExternalInput")
    wqk_d = nc.dram_tensor("wqk", (4, 128, 2 * DIM), F16, kind="ExternalInput")
    wv_d = nc.dram_tensor("wv", (4, 128, DIM), F16, kind="ExternalInput")
    wp_d = nc.dram_tensor("wp", (4, 128, DIM), F16, kind="ExternalInput")
    bqk_d = nc.dram_tensor("bqk", (128, 8), F32, kind="ExternalInput")
    bv_d = nc.dram_tensor("bv", (DIM,), F32, kind="ExternalInput")
    bp_d = nc.dram_tensor("bp", (DIM,), F32, kind="ExternalInput")
    tab_d = nc.dram_tensor("tab", (RPE, 128), F16, kind="ExternalInput")
    idx_d = nc.dram_tensor("idx", (128, _GQ * 8 * _NGATHER), I16,
                           kind="ExternalInput")
    mb_d = nc.dram_tensor("mb", (NKC, n_w * 2), F32, kind="ExternalInput")
    ident_d = nc.dram_tensor("ident", (128, 128), F16, kind="ExternalInput")
    out_d = nc.dram_tensor("out", (n_w, N, DIM), F32, kind="ExternalOutput")

    x16_d = nc.dram_tensor("x16", (n_w * N, DIM), F16)

    with tile.TileContext(nc) as tc, contextlib.ExitStack() as ctx:
        const = ctx.enter_context(tc.tile_pool(name="const", bufs=1))
        gpool = ctx.enter_context(tc.tile_pool(name="gather", bufs=2))
        xt_pool = ctx.enter_context(tc.tile_pool(
            name="xt", bufs=(3 if variant == "tune2" else 2)))
        qk_pool = ctx.enter_context(tc.tile_pool(
            name="qk", bufs=(3 if variant == "tune2" else 2)))
        v_pool = ctx.enter_context(tc.tile_pool(name="v", bufs=2))
        p_pool = ctx.enter_context(tc.tile_pool(
            name="p", bufs=(4 if variant in ("tune1", "tune2") else 3)))
        o_pool = ctx.enter_context(tc.tile_pool(
            name="o", bufs=(3 if variant == "tune2" else 2)))
        y_pool = ctx.enter_context(tc.tile_pool(name="y", bufs=3))
        rz_pool = ctx.enter_context(tc.tile_pool(
            name="rz", bufs=(4 if variant == "tune2" else 3)))
        ps_s = ctx.enter_context(tc.tile_pool(
            name="ps_s", bufs=(2 if variant == "bundle2" else 1), space="PSUM"))
        ps_a = ctx.enter_context(tc.tile_pool(name="ps_a", bufs=4, space="PSUM"))

        # ---------------- constants ----------------
        wqk_sb = const.tile([128, 4, 2 * DIM], F16)   # [c128, ci, o]  (q|k)
        wv_sb = const.tile([128, 4, DIM], F16)
        wp_sb = const.tile([128, 4, DIM], F16)
        nc.sync.dma_start(out=wqk_sb, in_=wqk_d[:].rearrange("a b c -> b a c"))
        nc.sync.dma_start(out=wv_sb, in_=wv_d[:].rearrange("a b c -> b a c"))
        nc.sync.dma_start(out=wp_sb, in_=wp_d[:].rearrange("a b c -> b a c"))
        ident_sb = const.tile([128, 128], F16)
        nc.sync.dma_start(out=ident_sb, in_=ident_d[:])
        ones_sb = const.tile([NKC, HD], F16)
        nc.vector.memset(ones_sb, 1.0)
        mb_sb = const.tile([NKC, n_w * 2], F32)
        nc.sync.dma_start(out=mb_sb, in_=mb_d[:])
        bqk_sb = const.tile([128, 8], F32)            # per-partition qk bias
        nc.sync.dma_start(out=bqk_sb, in_=bqk_d[:])
        bv_bc = const.tile([128, DIM], F32)           # broadcast rows
        nc.sync.dma_start(
            out=bv_bc, in_=bass.AP(tensor=bv_d[:].tensor, offset=0,
                                   ap=[[0, 128], [1, DIM]]))
        bp_bc = const.tile([128, DIM], F32)
        nc.sync.dma_start(
            out=bp_bc, in_=bass.AP(tensor=bp_d[:].tensor, offset=0,
                                   ap=[[0, 128], [1, DIM]]))
        idx_sb = const.tile([128, _GQ * 8 * _NGATHER], I16)
        nc.sync.dma_start(out=idx_sb, in_=idx_d[:])

        # gathered rpe bias: bias_sb[p, cq, h] = tab[idx[cq*128+p], h]
        bias_sb = const.tile([128, 2 * N, H], F16)
        for g in range(_NGATHER):
            g_sb = gpool.tile([128, _GQ, 128], F16, tag="gather")
            n_idx = _GQ * 128
            nc.gpsimd.dma_gather(
                out_ap=g_sb[:],
                in_ap=tab_d[:],
                idxs_ap=idx_sb[:, g * _GQ * 8:(g + 1) * _GQ * 8],
                num_idxs=n_idx,
                num_idxs_reg=n_idx,
                elem_size=128,
                single_packet=False,
            )
            nc.vector.tensor_copy(
                out=bias_sb[:, g * _GQ:(g + 1) * _GQ, :],
                in_=g_sb[:, :, 0:H],
            )

        # ---------------- main loop over 4-window groups ----------------
        for g in range(ngrp):
            tok0 = g * GW * N
            nc.gpsimd.dma_start(
                out=x16_d[tok0:tok0 + GW * N, :],
                in_=x_d[:].rearrange("w n c -> (w n) c")[tok0:tok0 + GW * N, :],
            )
            xt = xt_pool.tile([128, 4, GW * N], F16, tag="xt")
            for ci in range(4):
                nc.sync.dma_start_transpose(
                    out=xt[:, ci, :],
                    in_=x16_d[tok0:tok0 + GW * N, ci * 128:(ci + 1) * 128],
                )

            # Q^T / K^T  [o-chunk 128, tok] fp16
            qk_sb = qk_pool.tile([128, 8, GW * N], F16, tag="qk")
            for oc in range(8):
                for half in range(2):
                    mm_ps = ps_a.tile([128, 512], F32, tag="ps_a")
                    for ci in range(4):
                        nc.tensor.matmul(
                            mm_ps[:, 0:392],
                            lhsT=wqk_sb[:, ci, oc * 128:(oc + 1) * 128],
                            rhs=xt[:, ci, half * 392:(half + 1) * 392],
                            start=(ci == 0), stop=(ci == 3),
                        )
                    nc.any.tensor_scalar_add(
                        out=qk_sb[:, oc, half * 392:(half + 1) * 392],
                        in0=mm_ps[:, 0:392],
                        scalar1=bqk_sb[:, oc:oc + 1],
                    )

            # V natural  [tok-chunk 98, 512] fp16
            v_sb = v_pool.tile([NKC, GW, 2, DIM], F16, tag="v")
            for wi in range(GW):
                for tcn in range(2):
                    vv_ps = ps_a.tile([128, 512], F32, tag="ps_a")
                    for ci in range(4):
                        nc.tensor.matmul(
                            vv_ps[0:NKC, :],
                            lhsT=xt[:, ci,
                                    wi * N + tcn * NKC:wi * N + (tcn + 1) * NKC],
                            rhs=wv_sb[:, ci, :],
                            start=(ci == 0), stop=(ci == 3),
                        )
                    nc.vector.tensor_add(
                        out=v_sb[:, wi, tcn, :],
                        in0=vv_ps[0:NKC, :],
                        in1=bv_bc[0:NKC, :],
                    )

            # ---------------- attention per window ----------------
            for wi in range(GW):
                w_abs = g * GW + wi
                oT = o_pool.tile([128, 4, N], F16, tag="oT")
                for hg in range(4):
                    p_sb = p_pool.tile([NKC, 2, 4, N], F16, tag="p")
                    if variant != "bundle2":
                        s_ps = ps_s.tile([128, 4, 512], F32, tag="s")
                    for c in range(2):
                        if variant == "bundle2":
                            s_ps = ps_s.tile([128, 2, 512], F32, tag="s")
                        if "qk" in ablate and "bias" in ablate:
                            nc.tensor.matmul(
                                s_ps[0:NKC, 0, 0:32],
                                lhsT=ident_sb[0:NKC, 0:NKC],
                                rhs=bias_sb[0:NKC, 0:2, 0:16
                                            ].rearrange("p q h -> p (q h)"),
                                start=True, stop=True,
                            )
                        for i in range(4):               # head = 4*hg + i
                            if "qk" in ablate:
                                break
                            if variant == "bundle2":
                                s_out = s_ps[0:NKC, i // 2,
                                             (i % 2) * 196:(i % 2) * 196 + 196]
                                st = (i % 2 == 0)
                            else:
                                s_out = s_ps[0:NKC, i, c * 196:c * 196 + 196]
                                st = True
                            if variant == "tune2":
                                nc.tensor.matmul(
                                    s_ps[0:NKC, i, c * 196:c * 196 + 196],
                                    lhsT=ident_sb[0:NKC, 0:NKC],
                                    rhs=bias_sb[0:NKC, c * N:(c + 1) * N,
                                                4 * hg + i],
                                    start=True, stop=False,
                                )
                            nc.tensor.matmul(
                                s_out,
                                lhsT=qk_sb[32 * i:32 * (i + 1), 4 + hg,
                                           wi * N + c * NKC:
                                           wi * N + (c + 1) * NKC],
                                rhs=qk_sb[32 * i:32 * (i + 1), hg,
                                          wi * N:(wi + 1) * N],
                                start=(st and variant != "tune2"),
                                stop=(variant == "tune2"),
                                tile_position=(32 * i, 0),
                            )
                        if "bias" not in ablate and variant == "bundle2":
                            for pr in range(2):          # head pair
                                nc.tensor.matmul(
                                    s_ps[0:NKC, pr, 0:392],
                                    lhsT=ident_sb[0:NKC, 0:NKC],
                                    rhs=bias_sb[0:NKC, c * N:(c + 1) * N,
                                                4 * hg + 2 * pr:
                                                4 * hg + 2 * pr + 2
                                                ].rearrange("p q h -> p h q"),
                                    start=("qk" in ablate), stop=True,
                                )
                        elif "bias" not in ablate and variant != "tune2":
                            for i in range(4):           # rpe bias, K=98 each
                                h = 4 * hg + i
                                nc.tensor.matmul(
                                    s_ps[0:NKC, i, c * 196:c * 196 + 196],
                                    lhsT=ident_sb[0:NKC, 0:NKC],
                                    rhs=bias_sb[0:NKC, c * N:(c + 1) * N, h],
                                    start=("qk" in ablate), stop=True,
                                )
                        if variant == "batch":
                            continue                     # exps after all MMs
                        if variant == "bundle2":
                            exp_in = s_ps[0:NKC, :, 0:392]
                        else:
                            exp_in = s_ps[0:NKC, :, c * 196:c * 196 + 196]
                        nc.scalar.activation(
                            out=p_sb[:, c, :, :],
                            in_=exp_in,
                            func=mybir.ActivationFunctionType.Exp,
                            bias=mb_sb[:, 2 * w_abs + c:2 * w_abs + c + 1],
                            scale=1.0,
                        )
                    if variant == "batch":
                        for c in range(2):
                            nc.scalar.activation(
                                out=p_sb[:, c, :, :],
                                in_=s_ps[0:NKC, :, c * 196:c * 196 + 196],
                                func=mybir.ActivationFunctionType.Exp,
                                bias=mb_sb[:, 2 * w_abs + c:
                                           2 * w_abs + c + 1],
                                scale=1.0,
                            )
                    # PV + Z, col-packed over the 4 heads
                    o_ps = ps_a.tile([128, 512], F32, tag="ps_a")
                    z_ps = ps_a.tile([128, 512], F32, tag="ps_a")
                    for i in range(4):
                        h = 4 * hg + i
                        if "pv" in ablate and i == 0:
                            nc.tensor.matmul(
                                o_ps[0:32, 0:16],
                                lhsT=v_sb[:, wi, 0, 0:32],
                                rhs=p_sb[:, 0, 0, 0:16],
                                start=True, stop=True,
                            )
                        if "pv" not in ablate:
                            for c in range(2):
                                nc.tensor.matmul(
                                    o_ps[32 * i:32 * (i + 1), 0:N],
                                    lhsT=v_sb[:, wi, c, 32 * h:32 * (h + 1)],
                                    rhs=p_sb[:, c, i, :],
                                    start=(c == 0), stop=(c == 1),
                                    tile_position=(0, 32 * i),
                                )
                        if "z" in ablate and i == 0:
                            nc.tensor.matmul(
                                z_ps[0:32, 0:16],
                                lhsT=ones_sb[:, 0:32],
                                rhs=p_sb[:, 0, 0, 0:16],
                                start=True, stop=True,
                            )
                        if "z" not in ablate:
                            for c in range(2):
                                nc.tensor.matmul(
                                    z_ps[32 * i:32 * (i + 1), 0:N],
                                    lhsT=ones_sb[:],
                                    rhs=p_sb[:, c, i, :],
                                    start=(c == 0), stop=(c == 1),
                                    tile_position=(0, 32 * i),
                                )
                    rz = rz_pool.tile([128, N], F32, tag="rz")
                    if variant in ("tune1", "tune2"):
                        nc.vector.reciprocal(out=rz[:], in_=z_ps[:, 0:N])
                    else:
                        z_sb = rz_pool.tile([128, N], F32, tag="z")
                        nc.scalar.copy(out=z_sb[:], in_=z_ps[:, 0:N])
                        nc.vector.reciprocal_approx_fast(out=rz[:], in_=z_sb[:])
                    nc.vector.tensor_mul(
                        out=oT[:, hg, :], in0=o_ps[:, 0:N], in1=rz[:])

                # ---------------- proj ----------------
                for qc in range(2):
                    y_ps = ps_a.tile([128, 512], F32, tag="ps_a")
                    for hg in range(4):
                        nc.tensor.matmul(
                            y_ps[0:NKC, :],
                            lhsT=oT[:, hg, qc * NKC:(qc + 1) * NKC],
                            rhs=wp_sb[:, hg, :],
                            start=(hg == 0), stop=(hg == 3),
                        )
                    y_sb = y_pool.tile([NKC, DIM], F32, tag="y")
                    nc.vector.tensor_add(
                        out=y_sb[:], in0=y_ps[0:NKC, :], in1=bp_bc[0:NKC, :])
                    nc.sync.dma_start(
                        out=out_d[w_abs, qc * NKC:(qc + 1) * NKC, :],
                        in_=y_sb[:],
                    )
    nc.compile()
    return nc


def _build_v2(n_w=W, n_rep=1):
    """v2: RPE bias applied as exp(bias) multiply on DVE (host pre-exp'd
    table), S matmuls close their own accumulation. n_rep repeats the whole
    compute body inside one program (for slope timing)."""
    assert n_w % GW == 0
    ngrp = n_w // GW
    nc = Bacc("TRN2", target_bir_lowering=False)

    x_d = nc.dram_tensor("x", (n_w, N, DIM), F32, kind="ExternalInput")
    wqk_d = nc.dram_tensor("wqk", (4, 128, 2 * DIM), F16, kind="ExternalInput")
    wv_d = nc.dram_tensor("wv", (4, 128, DIM), F16, kind="ExternalInput")
    wp_d = nc.dram_tensor("wp", (4, 128, DIM), F16, kind="ExternalInput")
    bqk_d = nc.dram_tensor("bqk", (128, 8), F32, kind="ExternalInput")
    bv_d = nc.dram_tensor("bv", (DIM,), F32, kind="ExternalInput")
    bp_d = nc.dram_tensor("bp", (DIM,), F32, kind="ExternalInput")
    tab_d = nc.dram_tensor("tab", (RPE, 128), F16, kind="ExternalInput")
    idx_d = nc.dram_tensor("idx", (128, _GQ * 8 * _NGATHER), I16,
                           kind="ExternalInput")
    mb_d = nc.dram_tensor("mb", (NKC, n_w * 2), F32, kind="ExternalInput")
    ident_d = nc.dram_tensor("ident", (128, 128), F16, kind="ExternalInput")
    out_d = nc.dram_tensor("out", (n_w, N, DIM), F32, kind="ExternalOutput")

    x16_d = nc.dram_tensor("x16", (n_w * N, DIM), F16)

    with tile.TileContext(nc) as tc, contextlib.ExitStack() as ctx:
        const = ctx.enter_context(tc.tile_pool(name="const", bufs=1))
        gpool = ctx.enter_context(tc.tile_pool(name="gather", bufs=2))
        xt_pool = ctx.enter_context(tc.tile_pool(name="xt", bufs=2))
        qk_pool = ctx.enter_context(tc.tile_pool(name="qk", bufs=2))
        v_pool = ctx.enter_context(tc.tile_pool(name="v", bufs=2))
        praw_pool = ctx.enter_context(tc.tile_pool(name="praw", bufs=3))
        p_pool = ctx.enter_context(tc.tile_pool(name="p", bufs=4))
        o_pool = ctx.enter_context(tc.tile_pool(name="o", bufs=2))
        y_pool = ctx.enter_context(tc.tile_pool(name="y", bufs=3))
        rz_pool = ctx.enter_context(tc.tile_pool(name="rz", bufs=4))
        ps_s = ctx.enter_context(tc.tile_pool(name="ps_s", bufs=1, space="PSUM"))
        ps_a = ctx.enter_context(tc.tile_pool(name="ps_a", bufs=4, space="PSUM"))

        # ---------------- constants ----------------
        wqk_sb = const.tile([128, 4, 2 * DIM], F16)   # [c128, ci, o]  (q|k)
        wv_sb = const.tile([128, 4, DIM], F16)
        wp_sb = const.tile([128, 4, DIM], F16)
        nc.sync.dma_start(out=wqk_sb, in_=wqk_d[:].rearrange("a b c -> b a c"))
        nc.sync.dma_start(out=wv_sb, in_=wv_d[:].rearrange("a b c -> b a c"))
        nc.sync.dma_start(out=wp_sb, in_=wp_d[:].rearrange("a b c -> b a c"))
        ident_sb = const.tile([128, 128], F16)
        nc.sync.dma_start(out=ident_sb, in_=ident_d[:])
        ones_sb = const.tile([NKC, HD], F16)
        nc.vector.memset(ones_sb, 1.0)
        mb_sb = const.tile([NKC, n_w * 2], F32)
        nc.sync.dma_start(out=mb_sb, in_=mb_d[:])
        bqk_sb = const.tile([128, 8], F32)            # per-partition qk bias
        nc.sync.dma_start(out=bqk_sb, in_=bqk_d[:])
        bv_bc = const.tile([128, DIM], F32)           # broadcast rows
        nc.sync.dma_start(
            out=bv_bc, in_=bass.AP(tensor=bv_d[:].tensor, offset=0,
                                   ap=[[0, 128], [1, DIM]]))
        bp_bc = const.tile([128, DIM], F32)
        nc.sync.dma_start(
            out=bp_bc, in_=bass.AP(tensor=bp_d[:].tensor, offset=0,
                                   ap=[[0, 128], [1, DIM]]))
        idx_sb = const.tile([128, _GQ * 8 * _NGATHER], I16)
        nc.sync.dma_start(out=idx_sb, in_=idx_d[:])

        # gathered exp(rpe bias): eb2[p, h, cq] = exp_tab[idx[cq*128+p], h]
        eb2 = const.tile([128, H, 2 * N], F16)
        for g in range(_NGATHER):
            g_sb = gpool.tile([128, _GQ, 128], F16, tag="gather")
            n_idx = _GQ * 128
            nc.gpsimd.dma_gather(
                out_ap=g_sb[:],
                in_ap=tab_d[:],
                idxs_ap=idx_sb[:, g * _GQ * 8:(g + 1) * _GQ * 8],
                num_idxs=n_idx,
                num_idxs_reg=n_idx,
                elem_size=128,
                single_packet=False,
            )
            nc.vector.tensor_copy(
                out=eb2[:, :, g * _GQ:(g + 1) * _GQ],
                in_=g_sb[:, :, 0:H].rearrange("p q h -> p h q"),
            )

        for rep in range(n_rep):
            # ---------------- main loop over 4-window groups ----------------
            for g in range(ngrp):
                tok0 = g * GW * N
                nc.gpsimd.dma_start(
                    out=x16_d[tok0:tok0 + GW * N, :],
                    in_=x_d[:].rearrange(
                        "w n c -> (w n) c")[tok0:tok0 + GW * N, :],
                )
                xt = xt_pool.tile([128, 4, GW * N], F16, tag="xt")
                for ci in range(4):
                    nc.sync.dma_start_transpose(
                        out=xt[:, ci, :],
                        in_=x16_d[tok0:tok0 + GW * N, ci * 128:(ci + 1) * 128],
                    )

                # Q^T / K^T  [o-chunk 128, tok] fp16
                qk_sb = qk_pool.tile([128, 8, GW * N], F16, tag="qk")
                for oc in range(8):
                    for half in range(2):
                        mm_ps = ps_a.tile([128, 512], F32, tag="ps_a")
                        for ci in range(4):
                            nc.tensor.matmul(
                                mm_ps[:, 0:392],
                                lhsT=wqk_sb[:, ci, oc * 128:(oc + 1) * 128],
                                rhs=xt[:, ci, half * 392:(half + 1) * 392],
                                start=(ci == 0), stop=(ci == 3),
                            )
                        nc.any.tensor_scalar_add(
                            out=qk_sb[:, oc, half * 392:(half + 1) * 392],
                            in0=mm_ps[:, 0:392],
                            scalar1=bqk_sb[:, oc:oc + 1],
                        )

                # V natural  [tok-chunk 98, 512] fp16
                v_sb = v_pool.tile([NKC, GW, 2, DIM], F16, tag="v")
                for wi in range(GW):
                    for tcn in range(2):
                        vv_ps = ps_a.tile([128, 512], F32, tag="ps_a")
                        for ci in range(4):
                            nc.tensor.matmul(
                                vv_ps[0:NKC, :],
                                lhsT=xt[:, ci, wi * N + tcn * NKC:
                                        wi * N + (tcn + 1) * NKC],
                                rhs=wv_sb[:, ci, :],
                                start=(ci == 0), stop=(ci == 3),
                            )
                        nc.vector.tensor_add(
                            out=v_sb[:, wi, tcn, :],
                            in0=vv_ps[0:NKC, :],
                            in1=bv_bc[0:NKC, :],
                        )

                # ---------------- attention per window ----------------
                for wi in range(GW):
                    w_abs = g * GW + wi
                    oT = o_pool.tile([128, 4, N], F16, tag="oT")
                    for hg in range(4):
                        p_raw = praw_pool.tile([NKC, 2, 4, N], F16, tag="praw")
                        p_sb = p_pool.tile([NKC, 2, 4, N], F16, tag="p")
                        s_ps = ps_s.tile([128, 4, 512], F32, tag="s")
                        for c in range(2):
                            for i in range(4):           # head = 4*hg + i
                                nc.tensor.matmul(
                                    s_ps[0:NKC, i, c * 196:c * 196 + 196],
                                    lhsT=qk_sb[32 * i:32 * (i + 1), 4 + hg,
                                               wi * N + c * NKC:
                                               wi * N + (c + 1) * NKC],
                                    rhs=qk_sb[32 * i:32 * (i + 1), hg,
                                              wi * N:(wi + 1) * N],
                                    start=True, stop=True,
                                    tile_position=(32 * i, 0),
                                )
                            nc.scalar.activation(
                                out=p_raw[:, c, :, :],
                                in_=s_ps[0:NKC, :, c * 196:c * 196 + 196],
                                func=mybir.ActivationFunctionType.Exp,
                                bias=mb_sb[:, 2 * w_abs + c:2 * w_abs + c + 1],
                                scale=1.0,
                            )
                            nc.vector.tensor_mul(
                                out=p_sb[:, c, :, :],
                                in0=p_raw[:, c, :, :],
                                in1=eb2[0:NKC, 4 * hg:4 * hg + 4,
                                        c * N:(c + 1) * N],
                            )
                        # PV + Z, col-packed over the 4 heads
                        o_ps = ps_a.tile([128, 512], F32, tag="ps_a")
                        z_ps = ps_a.tile([128, 512], F32, tag="ps_a")
                        for i in range(4):
                            h = 4 * hg + i
                            for c in range(2):
                                nc.tensor.matmul(
                                    o_ps[32 * i:32 * (i + 1), 0:N],
                                    lhsT=v_sb[:, wi, c, 32 * h:32 * (h + 1)],
                                    rhs=p_sb[:, c, i, :],
                                    start=(c == 0), stop=(c == 1),
                                    tile_position=(0, 32 * i),
                                )
                            for c in range(2):
                                nc.tensor.matmul(
                                    z_ps[32 * i:32 * (i + 1), 0:N],
                                    lhsT=ones_sb[:],
                                    rhs=p_sb[:, c, i, :],
                                    start=(c == 0), stop=(c == 1),
                                    tile_position=(0, 32 * i),
                                )
                        rz = rz_pool.tile([128, N], F32, tag="rz")
                        nc.vector.reciprocal(out=rz[:], in_=z_ps[:, 0:N])
                        nc.vector.tensor_mul(
                            out=oT[:, hg, :], in0=o_ps[:, 0:N], in1=rz[:])

                    # ---------------- proj ----------------
                    for qc in range(2):
                        y_ps = ps_a.tile([128, 512], F32, tag="ps_a")
                        for hg in range(4):
                            nc.tensor.matmul(
                                y_ps[0:NKC, :],
                                lhsT=oT[:, hg, qc * NKC:(qc + 1) * NKC],
                                rhs=wp_sb[:, hg, :],
                                start=(hg == 0), stop=(hg == 3),
                            )
                        y_sb = y_pool.tile([NKC, DIM], F32, tag="y")
                        nc.vector.tensor_add(
                            out=y_sb[:], in0=y_ps[0:NKC, :],
                            in1=bp_bc[0:NKC, :])
                        nc.sync.dma_start(
                            out=out_d[w_abs, qc * NKC:(qc + 1) * NKC, :],
                            in_=y_sb[:],
                        )
    nc.compile()
    return nc


def _build_v3(n_w=W, n_rep=1, tr_f32=False, no_bcast=False, stage=6, sub=0):
    """v3: flipped PV with ones-augmented V — one matmul pass computes both
    O (natural, q-partitioned) and the softmax denominator Z (33rd column),
    normalize is a per-partition-scalar multiply, O is PE-transposed for
    proj. RPE bias applied as exp-table multiply on DVE (as v2).
    tr_f32: transpose O in f32 instead of fp16 (fp16 PSUM suspect).
    no_bcast: normalize via per-head tensor_scalar instead of stride-0."""
    assert n_w % GW == 0
    ngrp = n_w // GW
    nc = Bacc("TRN2", target_bir_lowering=False)

    x_d = nc.dram_tensor("x", (n_w, N, DIM), F32, kind="ExternalInput")
    wqk_d = nc.dram_tensor("wqk", (4, 128, 2 * DIM), F16, kind="ExternalInput")
    wv_d = nc.dram_tensor("wv", (4, 128, DIM), F16, kind="ExternalInput")
    wp_d = nc.dram_tensor("wp", (4, 128, DIM), F16, kind="ExternalInput")
    bqk_d = nc.dram_tensor("bqk", (128, 8), F32, kind="ExternalInput")
    bv_d = nc.dram_tensor("bv", (DIM,), F32, kind="ExternalInput")
    bp_d = nc.dram_tensor("bp", (DIM,), F32, kind="ExternalInput")
    ebh_d = nc.dram_tensor("ebh", (128, H, 2 * N), F16, kind="ExternalInput")
    mb_d = nc.dram_tensor("mb", (NKC, n_w * 2), F32, kind="ExternalInput")
    ident_d = nc.dram_tensor("ident", (128, 128), F16, kind="ExternalInput")
    out_d = nc.dram_tensor("out", (n_w, N, DIM), F32, kind="ExternalOutput")

    x16_d = nc.dram_tensor("x16", (n_w * N, DIM), F16)

    with tile.TileContext(nc) as tc, contextlib.ExitStack() as ctx:
        const = ctx.enter_context(tc.tile_pool(name="const", bufs=1))
        gpool = ctx.enter_context(tc.tile_pool(name="gather", bufs=2))
        xt_pool = ctx.enter_context(tc.tile_pool(name="xt", bufs=2))
        qk_pool = ctx.enter_context(tc.tile_pool(name="qk", bufs=2))
        v_pool = ctx.enter_context(tc.tile_pool(name="v", bufs=2))
        praw_pool = ctx.enter_context(tc.tile_pool(name="praw", bufs=4))
        p_pool = ctx.enter_context(tc.tile_pool(name="p", bufs=6))
        o_pool = ctx.enter_context(tc.tile_pool(name="o", bufs=2))
        ot_pool = ctx.enter_context(tc.tile_pool(name="ot", bufs=2))
        rz_pool = ctx.enter_context(tc.tile_pool(name="rz", bufs=4))
        y_pool = ctx.enter_context(tc.tile_pool(name="y", bufs=3))
        ps_s = ctx.enter_context(tc.tile_pool(name="ps_s", bufs=1,
                                              space="PSUM"))
        ps_og = ctx.enter_context(tc.tile_pool(name="ps_og", bufs=2,
                                               space="PSUM"))
        ps_a = ctx.enter_context(tc.tile_pool(name="ps_a", bufs=2,
                                              space="PSUM"))

        # ---------------- constants ----------------
        wqk_sb = const.tile([128, 4, 2 * DIM], F16)   # [c128, ci, o]  (q|k)
        wv_sb = const.tile([128, 4, DIM], F16)
        wp_sb = const.tile([128, 4, DIM], F16)
        nc.sync.dma_start(out=wqk_sb, in_=wqk_d[:].rearrange("a b c -> b a c"))
        nc.sync.dma_start(out=wv_sb, in_=wv_d[:].rearrange("a b c -> b a c"))
        nc.sync.dma_start(out=wp_sb, in_=wp_d[:].rearrange("a b c -> b a c"))
        ident_sb = const.tile([128, 128], F16)
        nc.sync.dma_start(out=ident_sb, in_=ident_d[:])
        if tr_f32:
            ident32_sb = const.tile([128, 128], F32)
            nc.vector.tensor_copy(out=ident32_sb, in_=ident_sb[:])
        mb_sb = const.tile([NKC, n_w * 2], F32)
        nc.sync.dma_start(out=mb_sb, in_=mb_d[:])
        bqk_sb = const.tile([128, 8], F32)            # per-partition qk bias
        nc.sync.dma_start(out=bqk_sb, in_=bqk_d[:])
        bv_bc = const.tile([128, DIM], F32)           # broadcast rows
        nc.sync.dma_start(
            out=bv_bc, in_=bass.AP(tensor=bv_d[:].tensor, offset=0,
                                   ap=[[0, 128], [1, DIM]]))
        bp_bc = const.tile([128, DIM], F32)
        nc.sync.dma_start(
            out=bp_bc, in_=bass.AP(tensor=bp_d[:].tensor, offset=0,
                                   ap=[[0, 128], [1, DIM]]))
        # host-gathered exp(rpe bias): eb2[p, h, cq] = exp_tab[idx[cq*128+p], h]
        eb2 = const.tile([128, H, 2 * N], F16)
        nc.sync.dma_start(out=eb2, in_=ebh_d[:])

        for rep in range(n_rep):
            # ------------- main loop over 4-window groups -------------
            for g in range(ngrp):
                tok0 = g * GW * N
                nc.gpsimd.dma_start(
                    out=x16_d[tok0:tok0 + GW * N, :],
                    in_=x_d[:].rearrange(
                        "w n c -> (w n) c")[tok0:tok0 + GW * N, :],
                )
                xt = xt_pool.tile([128, 4, GW * N], F16, tag="xt")
                for ci in range(4):
                    nc.sync.dma_start_transpose(
                        out=xt[:, ci, :],
                        in_=x16_d[tok0:tok0 + GW * N, ci * 128:(ci + 1) * 128],
                    )

                # Q^T / K^T  [o-chunk 128, tok] fp16
                qk_sb = qk_pool.tile([128, 8, GW * N], F16, tag="qk")
                for oc in range(8):
                    for half in range(2):
                        mm_ps = ps_a.tile([128, 512], F32, tag="ps_a")
                        for ci in range(4):
                            nc.tensor.matmul(
                                mm_ps[:, 0:392],
                                lhsT=wqk_sb[:, ci, oc * 128:(oc + 1) * 128],
                                rhs=xt[:, ci, half * 392:(half + 1) * 392],
                                start=(ci == 0), stop=(ci == 3),
                            )
                        nc.any.tensor_scalar_add(
                            out=qk_sb[:, oc, half * 392:(half + 1) * 392],
                            in0=mm_ps[:, 0:392],
                            scalar1=bqk_sb[:, oc:oc + 1],
                        )

                # V natural  [tok-chunk 98, h, 33] fp16 (col 32 = ones)
                v_sb = v_pool.tile([NKC, GW, 2, H, 33], F16, tag="v")
                nc.vector.memset(v_sb[:, :, :, :, 32:33], 1.0)
                for wi in range(GW):
                    for tcn in range(2):
                        vv_ps = ps_a.tile([128, 512], F32, tag="ps_a")
                        for ci in range(4):
                            nc.tensor.matmul(
                                vv_ps[0:NKC, :],
                                lhsT=xt[:, ci, wi * N + tcn * NKC:
                                        wi * N + (tcn + 1) * NKC],
                                rhs=wv_sb[:, ci, :],
                                start=(ci == 0), stop=(ci == 3),
                            )
                        nc.vector.tensor_add(
                            out=v_sb[:, wi, tcn, :, 0:32],
                            in0=vv_ps[0:NKC, :].rearrange(
                                "p (h d) -> p h d", h=H),
                            in1=bv_bc[0:NKC, :].rearrange(
                                "p (h d) -> p h d", h=H),
                        )

                # ---------------- attention per window ----------------
                for wi in range(GW):
                    w_abs = g * GW + wi
                    if stage < 2:
                        for qc in range(2):
                            y_sb = y_pool.tile([NKC, DIM], F32, tag="y")
                            nc.vector.memset(y_sb[:], 0.0)
                            nc.sync.dma_start(
                                out=out_d[w_abs, qc * NKC:(qc + 1) * NKC, :],
                                in_=y_sb[:])
                        continue
                    p_tiles = []
                    for hg in range(4):
                        p_raw = praw_pool.tile([NKC, 2, 4, N], F16,
                                               tag="praw")
                        p_sb = p_pool.tile([NKC, 2, 4, N], F16, tag="p")
                        p_tiles.append(p_sb)
                        s_ps = ps_s.tile([128, 4, 512], F32, tag="s")
                        for c in range(2):
                            for i in range(4):        # head = 4*hg + i
                                nc.tensor.matmul(
                                    s_ps[0:NKC, i, c * 196:c * 196 + 196],
                                    lhsT=qk_sb[32 * i:32 * (i + 1), 4 + hg,
                                               wi * N + c * NKC:
                                               wi * N + (c + 1) * NKC],
                                    rhs=qk_sb[32 * i:32 * (i + 1), hg,
                                              wi * N:(wi + 1) * N],
                                    start=True, stop=True,
                                    tile_position=(32 * i, 0),
                                )
                            nc.scalar.activation(
                                out=p_raw[:, c, :, :],
                                in_=s_ps[0:NKC, :, c * 196:c * 196 + 196],
                                func=mybir.ActivationFunctionType.Exp,
                                bias=mb_sb[:, 2 * w_abs + c:
                                           2 * w_abs + c + 1],
                                scale=1.0,
                            )
                            eb_eng = nc.gpsimd if hg % 2 else nc.vector
                            eb_eng.tensor_mul(
                                out=p_sb[:, c, :, :],
                                in0=p_raw[:, c, :, :],
                                in1=eb2[0:NKC, 4 * hg:4 * hg + 4,
                                        c * N:(c + 1) * N],
                            )

                    if stage < 3:
                        for qc in range(2):
                            y_sb = y_pool.tile([NKC, DIM], F32, tag="y")
                            nc.vector.memset(y_sb[:], 0.0)
                            nc.sync.dma_start(
                                out=out_d[w_abs, qc * NKC:(qc + 1) * NKC, :],
                                in_=y_sb[:])
                        continue
                    # PV+Z flipped: og[q, (h8, 33)] per (qb, grp)
                    o_sb = o_pool.tile([NKC, 2, DIM], F16, tag="o")
                    for qb in range(2):
                        for grp in range(2):
                            og = ps_og.tile([NKC, 512], F32, tag="og")
                            for h8 in range(8):
                                h = grp * 8 + h8
                                for c in range(2):
                                    nc.tensor.matmul(
                                        og[0:NKC, 33 * h8:33 * h8 + 33],
                                        lhsT=p_tiles[h // 4][
                                            :, c, h % 4,
                                            qb * NKC:(qb + 1) * NKC],
                                        rhs=v_sb[:, wi, c, h, :],
                                        start=(c == 0), stop=(c == 1),
                                    )
                            if stage < 4:
                                y_sb = y_pool.tile([NKC, DIM], F32, tag="y")
                                nc.vector.tensor_copy(
                                    out=y_sb[:, 0:264], in_=og[0:NKC, 0:264])
                                nc.vector.memset(y_sb[:, 264:512], 0.0)
                                nc.sync.dma_start(
                                    out=out_d[w_abs,
                                              qb * NKC:(qb + 1) * NKC, :],
                                    in_=y_sb[:])
                                continue
                            ogv = og[0:NKC, 0:264].rearrange(
                                "p (h d) -> p h d", d=33)
                            rz = rz_pool.tile([NKC, 8], F32, tag="rz")
                            nc.vector.reciprocal(out=rz[:], in_=ogv[:, :, 32])
                            if no_bcast:
                                for h8 in range(8):
                                    nc.vector.tensor_scalar_mul(
                                        out=o_sb[:, qb,
                                                 grp * 256 + 32 * h8:
                                                 grp * 256 + 32 * h8 + 32],
                                        in0=ogv[:, h8, 0:32],
                                        scalar1=rz[:, h8:h8 + 1],
                                    )
                            else:
                                nc.vector.tensor_mul(
                                    out=o_sb[:, qb, grp * 256:grp * 256 + 256
                                             ].rearrange(
                                                 "p (h d) -> p h d", h=8),
                                    in0=ogv[:, :, 0:32],
                                    in1=rz[:].broadcast_to((NKC, 8, 32)),
                                )

                    if stage < 4:
                        continue
                    if stage < 5:
                        for qc in range(2):
                            y_sb = y_pool.tile([NKC, DIM], F32, tag="y")
                            nc.vector.tensor_copy(
                                out=y_sb[:], in_=o_sb[:, qc, :])
                            nc.sync.dma_start(
                                out=out_d[w_abs, qc * NKC:(qc + 1) * NKC, :],
                                in_=y_sb[:])
                        continue
                    # transpose O -> O^T and proj
                    ot_sb = ot_pool.tile([128, 4, 2, NKC], F16, tag="ot")
                    if tr_f32:
                        o32_sb = o_pool.tile([NKC, 2, DIM], F32, tag="o32")
                        nc.scalar.activation(
                            out=o32_sb[:], in_=o_sb[:],
                            func=mybir.ActivationFunctionType.Copy)
                    for cc in range(4):
                        for qb in range(2):
                            if tr_f32:
                                ot_ps = ps_a.tile([128, 512], F32,
                                                  tag="ps_a")
                                nc.tensor.transpose(
                                    out=ot_ps[:, 0:NKC],
                                    in_=o32_sb[:, qb,
                                               cc * 128:(cc + 1) * 128],
                                    identity=ident32_sb[0:NKC, 0:NKC],
                                )
                            else:
                                ot_ps = ps_a.tile([128, 1024], F16,
                                                  tag="ps_a")
                                nc.tensor.transpose(
                                    out=ot_ps[:, 0:NKC],
                                    in_=o_sb[:, qb, cc * 128:(cc + 1) * 128],
                                    identity=ident_sb[0:NKC, 0:NKC],
                                )
                            nc.vector.tensor_copy(
                                out=ot_sb[:, cc, qb, :],
                                in_=ot_ps[:, 0:NKC],
                            )
                    if stage < 6:
                        for qc in range(2):
                            y_sb = y_pool.tile([NKC, DIM], F32, tag="y")
                            nc.vector.tensor_copy(
                                out=y_sb[:],
                                in_=ot_sb[:, :, qc, :].rearrange(
                                    "p a b -> p (a b)")[0:NKC, 0:DIM])
                            nc.sync.dma_start(
                                out=out_d[w_abs, qc * NKC:(qc + 1) * NKC, :],
                                in_=y_sb[:])
                        continue
                    for qc in range(2):
                        y_ps = ps_a.tile([128, 512], F32, tag="ps_a")
                        for cc in range(4):
                            nc.tensor.matmul(
                                y_ps[0:NKC, :],
                                lhsT=ot_sb[:, cc, qc, :],
                                rhs=wp_sb[:, cc, :],
                                start=(cc == 0), stop=(cc == 3),
                            )
                        y_sb = y_pool.tile([NKC, DIM], F32, tag="y")
                        nc.vector.tensor_add(
                            out=y_sb[:], in0=y_ps[0:NKC, :],
                            in1=bp_bc[0:NKC, :])
                        nc.sync.dma_start(
                            out=out_d[w_abs, qc * NKC:(qc + 1) * NKC, :],
                            in_=y_sb[:],
                        )
    nc.compile()
    return nc


def _host_prep(x, rpe_index, mask, qkv_w, qkv_b, proj_w, proj_b, rpe_table,
               n_w=W, n_cores=NCORES, exp_tab=True):
    """Shard + layout/dtype prep (numpy only). Returns per-core input maps."""
    x = np.asarray(x, dtype=np.float32)
    rpe_index = np.asarray(rpe_index).astype(np.int64)
    mask = np.asarray(mask).astype(np.int32)
    qkv_w = np.asarray(qkv_w, dtype=np.float32)
    qkv_b = np.asarray(qkv_b, dtype=np.float32)
    proj_w = np.asarray(proj_w, dtype=np.float32)
    proj_b = np.asarray(proj_b, dtype=np.float32)
    rpe_table = np.asarray(rpe_table, dtype=np.float32)

    scale = HD ** -0.5
    wq = qkv_w[0:DIM] * scale
    wk = qkv_w[DIM:2 * DIM]
    wv = qkv_w[2 * DIM:3 * DIM]
    wqk_t = np.concatenate([wq, wk], axis=0).T.astype(np.float16)  # [c, 1024]
    wv_t = wv.T.astype(np.float16)                                 # [c, 512]
    wp_t = proj_w.T.astype(np.float16)                             # [c, 512]
    wqk_t = np.ascontiguousarray(wqk_t.reshape(4, 128, 2 * DIM))
    wv_t = np.ascontiguousarray(wv_t.reshape(4, 128, DIM))
    wp_t = np.ascontiguousarray(wp_t.reshape(4, 128, DIM))

    bqk = np.concatenate([qkv_b[0:DIM] * scale, qkv_b[DIM:2 * DIM]])
    bqk_pp = np.ascontiguousarray(
        bqk.reshape(8, 128).T.astype(np.float32))                  # [128, 8]
    bv = qkv_b[2 * DIM:3 * DIM].astype(np.float32)

    tab = np.zeros((RPE, 128), dtype=np.float16)
    tab_vals = np.exp(rpe_table) if exp_tab else rpe_table
    tab[:, 0:H] = tab_vals.astype(np.float16)
    tab16 = np.ascontiguousarray(tab[:, 0:H])

    # gather index stream: position j = cq*128 + p ; cq = c*196+q ; k = 98c+p
    cq = np.arange(2 * N)
    c = cq // N
    q = cq % N
    p = np.arange(128)
    k = (NKC * c)[:, None] + p[None, :]                            # [392, 128]
    valid = p[None, :] < NKC
    j_idx = np.where(valid, rpe_index[q[:, None] * N + np.minimum(k, N - 1)], 0)
    j_idx = j_idx.reshape(-1).astype(np.int16)
    n_per = _GQ * 128
    idx_w = np.zeros((16, (n_per // 16) * _NGATHER), dtype=np.int16)
    for gch in range(_NGATHER):
        blk = j_idx[gch * n_per:(gch + 1) * n_per].reshape(n_per // 16, 16).T
        idx_w[:, gch * (n_per // 16):(gch + 1) * (n_per // 16)] = blk
    idx_w = np.ascontiguousarray(np.tile(idx_w, (8, 1)))           # [128, .]

    # host-gathered bias table: ebh[p, h, cq] = tab[j_idx[cq, p], h]
    j2 = j_idx.reshape(2 * N, 128)                                 # [cq, p]
    ebh = np.ascontiguousarray(
        tab[j2.astype(np.int64), 0:H].transpose(1, 2, 0))          # [p, h, cq]

    ident = np.eye(128, dtype=np.float16)

    in_maps = []
    for core in range(n_cores):
        xs = x[core * n_w:(core + 1) * n_w]
        ms = mask[core * n_w:(core + 1) * n_w]
        mbv = np.where(ms.astype(bool), EXP_SHIFT, MASK_NEG).astype(np.float32)
        mb = np.zeros((NKC, n_w * 2), dtype=np.float32)
        for wi in range(n_w):
            for cc in range(2):
                mb[:, 2 * wi + cc] = mbv[wi, cc * NKC:(cc + 1) * NKC]
        in_maps.append({
            "x": np.ascontiguousarray(xs),
            "wqk": wqk_t, "wv": wv_t, "wp": wp_t,
            "bqk": bqk_pp, "bv": bv, "bp": proj_b.astype(np.float32),
            "tab": tab, "ebh": ebh, "idx": idx_w,
            "mb": np.ascontiguousarray(mb),
            "ident": ident,
        })
    return in_maps


_NC_CACHE = {}
_BUILDER = _build_v3


def kernel(x, rpe_index, mask, qkv_w, qkv_b, proj_w, proj_b, rpe_table,
           _trace=False):
    from concourse.bass_utils import run_bass_kernel_spmd
    in_maps = _host_prep(x, rpe_index, mask, qkv_w, qkv_b, proj_w, proj_b,
                         rpe_table)
    if "nc" not in _NC_CACHE:
        _NC_CACHE["nc"] = _BUILDER()
    nc = _NC_CACHE["nc"]
    try:
        res = run_bass_kernel_spmd(nc, in_maps, core_ids=list(range(NCORES)),
                                   trace=_trace)
    except ModuleNotFoundError:
        # axon NTFF profiling hook unavailable in this container
        res = run_bass_kernel_spmd(nc, in_maps, core_ids=list(range(NCORES)),
                                   trace=False)
    kernel.last_results = res
    out = np.concatenate([r["out"] for r in res.results], axis=0)
    return out.reshape(B, N, DIM).astype(np.float32)



# revision 43
# speedup vs baseline: 1.2053x; 1.2053x over previous
"""Trainium2 Bass kernel for nn_Attention_4045859193206 (Swin-style window
attention with relative position bias + key masking).

Contract: kernel(**inputs) takes FULL inputs (B=128 windows), shards the batch
across 8 NeuronCores (16 windows each), runs one SPMD Bass kernel, returns the
FULL (128, 196, 512) float32 output.

Self-contained: hardcodes all shapes; no sibling imports.

Design v3 (per core, W=16 windows) — PE streaming cost is the bound, so all
work that does not need the systolic array is moved off it:
  - x cast fp32->fp16 on device (DRAM->DRAM DMA cast), then DMA-transposed to
    x^T [c, tok] in SBUF (4-window groups: 784 tokens, multiple of 16).
  - QKV: Q^T/K^T in transposed form ([o,tok], fp16, q pre-scaled via host-
    scaled weights); V in natural form [tok, h, 33] with a 33rd ones column
    per head appended.
  - S^T = K^T-lhsT matmuls, 4 heads row-packed via tile_position, one PSUM
    bank per PE tile position (matmuls at different tile positions must not
    share a PSUM bank — HW constraint found the hard way).
  - softmax numerator: P = exp(S + mask_bias - 4) on ScalarE (mask as
    per-partition bias AP; -4 cancels in normalization), then multiplied by
    the host-gathered exp(RPE bias) table (eb2 [k, h, q] fp16) on DVE/Pool
    (split by head group) — no PE cycles for the bias at all.
  - PV+Z in ONE flipped matmul pass: out[q-band, (h,33)] with lhsT = P chunk
    [k, q-band], rhs = [V_h | 1] [k, 33].  Column 32 of each head IS the
    softmax denominator Z, and the output lands q-partitioned so the
    normalization is a per-partition reciprocal + broadcast multiply
    (stride-0 free-dim AP), writing natural-layout O fp16.
  - O is PE-transposed (8 x [98,128] fp16 transposes, ~784 PE cycles/window)
    back to O^T for the projection; proj bias added during the final
    PSUM->SBUF pass, DMA out.
  - exp(bias) gather is done on the HOST (rpe_table/rpe_index are host
    visible): eb2 = exp(rpe_table)[rpe_index] shipped as a 1.6 MB input,
    replacing a 12.8 MB on-device dma_gather.
PE streaming per window: S 6272 + PV/Z 2112 + transpose 784 + QK 6272 +
V 4096 + proj 4096 ~= 23.6k cycles; model PE busy 160 us/core, measured
~115-130 us on hardware (baseline 220 us).
"""

import contextlib
import numpy as np

import concourse.bass as bass
import concourse.mybir as mybir
import concourse.tile as tile
from concourse.bacc import Bacc

# ---------------------------------------------------------------- constants
B, N, DIM, H = 128, 196, 512, 16
HD = DIM // H                     # 32
RPE = 729                         # (2*14-1)^2
NCORES = 8
W = B // NCORES                   # 16 windows per core
NKC = 98                          # k-chunk (2 chunks of 98 = 196)
GW = 4                            # windows per qkv group (4*196=784 tokens)
F16 = mybir.dt.float16
F32 = mybir.dt.float32
I16 = mybir.dt.int16
EXP_SHIFT = -4.0                  # exp(s-4): fp16 headroom; cancels in softmax
MASK_NEG = -1e9
_GQ = 98                          # (c,q) positions per gather chunk
_NGATHER = 4                      # 4 chunks of 98 positions = 392


def _build_nc(n_w=W, ablate=frozenset(), variant="base"):
    """Build the per-core Bass program for n_w windows.
    ablate: subset of {'z','bias','qk','pv'} - drop those matmuls (timing expts).
    variant: 'base' or 'bundle2' (2-head bias bundling + s_ps 2 banks x 2 bufs)."""
    assert n_w % GW == 0
    ngrp = n_w // GW
    nc = Bacc("TRN2", target_bir_lowering=False)

    x_d = nc.dram_tensor("x", (n_w, N, DIM), F32, kind="ExternalInput")
    wqk_d = nc.dram_tensor("wqk", (4, 128, 2 * DIM), F16, kind="ExternalInput")
    wv_d = nc.dram_tensor("wv", (4, 128, DIM), F16, kind="ExternalInput")
    wp_d = nc.dram_tensor("wp", (4, 128, DIM), F16, kind="ExternalInput")
    bqk_d = nc.dram_tensor("bqk", (128, 8), F32, kind="ExternalInput")
    bv_d = nc.dram_tensor("bv", (DIM,), F32, kind="ExternalInput")
    bp_d = nc.dram_tensor("bp", (DIM,), F32, kind="ExternalInput")
    tab_d = nc.dram_tensor("tab", (RPE, 128), F16, kind="ExternalInput")
    idx_d = nc.dram_tensor("idx", (128, _GQ * 8 * _NGATHER), I16,
                           kind="ExternalInput")
    mb_d = nc.dram_tensor("mb", (NKC, n_w * 2), F32, kind="ExternalInput")
    ident_d = nc.dram_tensor("ident", (128, 128), F16, kind="ExternalInput")
    out_d = nc.dram_tensor("out", (n_w, N, DIM), F32, kind="ExternalOutput")

    x16_d = nc.dram_tensor("x16", (n_w * N, DIM), F16)

    with tile.TileContext(nc) as tc, contextlib.ExitStack() as ctx:
        const = ctx.enter_context(tc.tile_pool(name="const", bufs=1))
        gpool = ctx.enter_context(tc.tile_pool(name="gather", bufs=2))
        xt_pool = ctx.enter_context(tc.tile_pool(
            name="xt", bufs=(3 if variant == "tune2" else 2)))
        qk_pool = ctx.enter_context(tc.tile_pool(
            name="qk", bufs=(3 if variant == "tune2" else 2)))
        v_pool = ctx.enter_context(tc.tile_pool(name="v", bufs=2))
        p_pool = ctx.enter_context(tc.tile_pool(
            name="p", bufs=(4 if variant in ("tune1", "tune2") else 3)))
        o_pool = ctx.enter_context(tc.tile_pool(
            name="o", bufs=(3 if variant == "tune2" else 2)))
        y_pool = ctx.enter_context(tc.tile_pool(name="y", bufs=3))
        rz_pool = ctx.enter_context(tc.tile_pool(
            name="rz", bufs=(4 if variant == "tune2" else 3)))
        ps_s = ctx.enter_context(tc.tile_pool(
            name="ps_s", bufs=(2 if variant == "bundle2" else 1), space="PSUM"))
        ps_a = ctx.enter_context(tc.tile_pool(name="ps_a", bufs=4, space="PSUM"))

        # ---------------- constants ----------------
        wqk_sb = const.tile([128, 4, 2 * DIM], F16)   # [c128, ci, o]  (q|k)
        wv_sb = const.tile([128, 4, DIM], F16)
        wp_sb = const.tile([128, 4, DIM], F16)
        nc.sync.dma_start(out=wqk_sb, in_=wqk_d[:].rearrange("a b c -> b a c"))
        nc.sync.dma_start(out=wv_sb, in_=wv_d[:].rearrange("a b c -> b a c"))
        nc.sync.dma_start(out=wp_sb, in_=wp_d[:].rearrange("a b c -> b a c"))
        ident_sb = const.tile([128, 128], F16)
        nc.sync.dma_start(out=ident_sb, in_=ident_d[:])
        ones_sb = const.tile([NKC, HD], F16)
        nc.vector.memset(ones_sb, 1.0)
        mb_sb = const.tile([NKC, n_w * 2], F32)
        nc.sync.dma_start(out=mb_sb, in_=mb_d[:])
        bqk_sb = const.tile([128, 8], F32)            # per-partition qk bias
        nc.sync.dma_start(out=bqk_sb, in_=bqk_d[:])
        bv_bc = const.tile([128, DIM], F32)           # broadcast rows
        nc.sync.dma_start(
            out=bv_bc, in_=bass.AP(tensor=bv_d[:].tensor, offset=0,
                                   ap=[[0, 128], [1, DIM]]))
        bp_bc = const.tile([128, DIM], F32)
        nc.sync.dma_start(
            out=bp_bc, in_=bass.AP(tensor=bp_d[:].tensor, offset=0,
                                   ap=[[0, 128], [1, DIM]]))
        idx_sb = const.tile([128, _GQ * 8 * _NGATHER], I16)
        nc.sync.dma_start(out=idx_sb, in_=idx_d[:])

        # gathered rpe bias: bias_sb[p, cq, h] = tab[idx[cq*128+p], h]
        bias_sb = const.tile([128, 2 * N, H], F16)
        for g in range(_NGATHER):
            g_sb = gpool.tile([128, _GQ, 128], F16, tag="gather")
            n_idx = _GQ * 128
            nc.gpsimd.dma_gather(
                out_ap=g_sb[:],
                in_ap=tab_d[:],
                idxs_ap=idx_sb[:, g * _GQ * 8:(g + 1) * _GQ * 8],
                num_idxs=n_idx,
                num_idxs_reg=n_idx,
                elem_size=128,
                single_packet=False,
            )
            nc.vector.tensor_copy(
                out=bias_sb[:, g * _GQ:(g + 1) * _GQ, :],
                in_=g_sb[:, :, 0:H],
            )

        # ---------------- main loop over 4-window groups ----------------
        for g in range(ngrp):
            tok0 = g * GW * N
            nc.gpsimd.dma_start(
                out=x16_d[tok0:tok0 + GW * N, :],
                in_=x_d[:].rearrange("w n c -> (w n) c")[tok0:tok0 + GW * N, :],
            )
            xt = xt_pool.tile([128, 4, GW * N], F16, tag="xt")
            for ci in range(4):
                nc.sync.dma_start_transpose(
                    out=xt[:, ci, :],
                    in_=x16_d[tok0:tok0 + GW * N, ci * 128:(ci + 1) * 128],
                )

            # Q^T / K^T  [o-chunk 128, tok] fp16
            qk_sb = qk_pool.tile([128, 8, GW * N], F16, tag="qk")
            for oc in range(8):
                for half in range(2):
                    mm_ps = ps_a.tile([128, 512], F32, tag="ps_a")
                    for ci in range(4):
                        nc.tensor.matmul(
                            mm_ps[:, 0:392],
                            lhsT=wqk_sb[:, ci, oc * 128:(oc + 1) * 128],
                            rhs=xt[:, ci, half * 392:(half + 1) * 392],
                            start=(ci == 0), stop=(ci == 3),
                        )
                    nc.any.tensor_scalar_add(
                        out=qk_sb[:, oc, half * 392:(half + 1) * 392],
                        in0=mm_ps[:, 0:392],
                        scalar1=bqk_sb[:, oc:oc + 1],
                    )

            # V natural  [tok-chunk 98, 512] fp16
            v_sb = v_pool.tile([NKC, GW, 2, DIM], F16, tag="v")
            for wi in range(GW):
                for tcn in range(2):
                    vv_ps = ps_a.tile([128, 512], F32, tag="ps_a")
                    for ci in range(4):
                        nc.tensor.matmul(
                            vv_ps[0:NKC, :],
                            lhsT=xt[:, ci,
                                    wi * N + tcn * NKC:wi * N + (tcn + 1) * NKC],
                            rhs=wv_sb[:, ci, :],
                            start=(ci == 0), stop=(ci == 3),
                        )
                    nc.vector.tensor_add(
                        out=v_sb[:, wi, tcn, :],
                        in0=vv_ps[0:NKC, :],
                        in1=bv_bc[0:NKC, :],
                    )

            # ---------------- attention per window ----------------
            for wi in range(GW):
                w_abs = g * GW + wi
                oT = o_pool.tile([128, 4, N], F16, tag="oT")
                for hg in range(4):
                    p_sb = p_pool.tile([NKC, 2, 4, N], F16, tag="p")
                    if variant != "bundle2":
                        s_ps = ps_s.tile([128, 4, 512], F32, tag="s")
                    for c in range(2):
                        if variant == "bundle2":
                            s_ps = ps_s.tile([128, 2, 512], F32, tag="s")
                        if "qk" in ablate and "bias" in ablate:
                            nc.tensor.matmul(
                                s_ps[0:NKC, 0, 0:32],
                                lhsT=ident_sb[0:NKC, 0:NKC],
                                rhs=bias_sb[0:NKC, 0:2, 0:16
                                            ].rearrange("p q h -> p (q h)"),
                                start=True, stop=True,
                            )
                        for i in range(4):               # head = 4*hg + i
                            if "qk" in ablate:
                                break
                            if variant == "bundle2":
                                s_out = s_ps[0:NKC, i // 2,
                                             (i % 2) * 196:(i % 2) * 196 + 196]
                                st = (i % 2 == 0)
                            else:
                                s_out = s_ps[0:NKC, i, c * 196:c * 196 + 196]
                                st = True
                            if variant == "tune2":
                                nc.tensor.matmul(
                                    s_ps[0:NKC, i, c * 196:c * 196 + 196],
                                    lhsT=ident_sb[0:NKC, 0:NKC],
                                    rhs=bias_sb[0:NKC, c * N:(c + 1) * N,
                                                4 * hg + i],
                                    start=True, stop=False,
                                )
                            nc.tensor.matmul(
                                s_out,
                                lhsT=qk_sb[32 * i:32 * (i + 1), 4 + hg,
                                           wi * N + c * NKC:
                                           wi * N + (c + 1) * NKC],
                                rhs=qk_sb[32 * i:32 * (i + 1), hg,
                                          wi * N:(wi + 1) * N],
                                start=(st and variant != "tune2"),
                                stop=(variant == "tune2"),
                                tile_position=(32 * i, 0),
                            )
                        if "bias" not in ablate and variant == "bundle2":
                            for pr in range(2):          # head pair
                                nc.tensor.matmul(
                                    s_ps[0:NKC, pr, 0:392],
                                    lhsT=ident_sb[0:NKC, 0:NKC],
                                    rhs=bias_sb[0:NKC, c * N:(c + 1) * N,
                                                4 * hg + 2 * pr:
                                                4 * hg + 2 * pr + 2
                                                ].rearrange("p q h -> p h q"),
                                    start=("qk" in ablate), stop=True,
                                )
                        elif "bias" not in ablate and variant != "tune2":
                            for i in range(4):           # rpe bias, K=98 each
                                h = 4 * hg + i
                                nc.tensor.matmul(
                                    s_ps[0:NKC, i, c * 196:c * 196 + 196],
                                    lhsT=ident_sb[0:NKC, 0:NKC],
                                    rhs=bias_sb[0:NKC, c * N:(c + 1) * N, h],
                                    start=("qk" in ablate), stop=True,
                                )
                        if variant == "batch":
                            continue                     # exps after all MMs
                        if variant == "bundle2":
                            exp_in = s_ps[0:NKC, :, 0:392]
                        else:
                            exp_in = s_ps[0:NKC, :, c * 196:c * 196 + 196]
                        nc.scalar.activation(
                            out=p_sb[:, c, :, :],
                            in_=exp_in,
                            func=mybir.ActivationFunctionType.Exp,
                            bias=mb_sb[:, 2 * w_abs + c:2 * w_abs + c + 1],
                            scale=1.0,
                        )
                    if variant == "batch":
                        for c in range(2):
                            nc.scalar.activation(
                                out=p_sb[:, c, :, :],
                                in_=s_ps[0:NKC, :, c * 196:c * 196 + 196],
                                func=mybir.ActivationFunctionType.Exp,
                                bias=mb_sb[:, 2 * w_abs + c:
                                           2 * w_abs + c + 1],
                                scale=1.0,
                            )
                    # PV + Z, col-packed over the 4 heads
                    o_ps = ps_a.tile([128, 512], F32, tag="ps_a")
                    z_ps = ps_a.tile([128, 512], F32, tag="ps_a")
                    for i in range(4):
                        h = 4 * hg + i
                        if "pv" in ablate and i == 0:
                            nc.tensor.matmul(
                                o_ps[0:32, 0:16],
                                lhsT=v_sb[:, wi, 0, 0:32],
                                rhs=p_sb[:, 0, 0, 0:16],
                                start=True, stop=True,
                            )
                        if "pv" not in ablate:
                            for c in range(2):
                                nc.tensor.matmul(
                                    o_ps[32 * i:32 * (i + 1), 0:N],
                                    lhsT=v_sb[:, wi, c, 32 * h:32 * (h + 1)],
                                    rhs=p_sb[:, c, i, :],
                                    start=(c == 0), stop=(c == 1),
                                    tile_position=(0, 32 * i),
                                )
                        if "z" in ablate and i == 0:
                            nc.tensor.matmul(
                                z_ps[0:32, 0:16],
                                lhsT=ones_sb[:, 0:32],
                                rhs=p_sb[:, 0, 0, 0:16],
                                start=True, stop=True,
                            )
                        if "z" not in ablate:
                            for c in range(2):
                                nc.tensor.matmul(
                                    z_ps[32 * i:32 * (i + 1), 0:N],
                                    lhsT=ones_sb[:],
                                    rhs=p_sb[:, c, i, :],
                                    start=(c == 0), stop=(c == 1),
                                    tile_position=(0, 32 * i),
                                )
                    rz = rz_pool.tile([128, N], F32, tag="rz")
                    if variant in ("tune1", "tune2"):
                        nc.vector.reciprocal(out=rz[:], in_=z_ps[:, 0:N])
                    else:
                        z_sb = rz_pool.tile([128, N], F32, tag="z")
                        nc.scalar.copy(out=z_sb[:], in_=z_ps[:, 0:N])
                        nc.vector.reciprocal_approx_fast(out=rz[:], in_=z_sb[:])
                    nc.vector.tensor_mul(
                        out=oT[:, hg, :], in0=o_ps[:, 0:N], in1=rz[:])

                # ---------------- proj ----------------
                for qc in range(2):
                    y_ps = ps_a.tile([128, 512], F32, tag="ps_a")
                    for hg in range(4):
                        nc.tensor.matmul(
                            y_ps[0:NKC, :],
                            lhsT=oT[:, hg, qc * NKC:(qc + 1) * NKC],
                            rhs=wp_sb[:, hg, :],
                            start=(hg == 0), stop=(hg == 3),
                        )
                    y_sb = y_pool.tile([NKC, DIM], F32, tag="y")
                    nc.vector.tensor_add(
                        out=y_sb[:], in0=y_ps[0:NKC, :], in1=bp_bc[0:NKC, :])
                    nc.sync.dma_start(
                        out=out_d[w_abs, qc * NKC:(qc + 1) * NKC, :],
                        in_=y_sb[:],
                    )
    nc.compile()
    return nc


def _build_v2(n_w=W, n_rep=1):
    """v2: RPE bias applied as exp(bias) multiply on DVE (host pre-exp'd
    table), S matmuls close their own accumulation. n_rep repeats the whole
    compute body inside one program (for slope timing)."""
    assert n_w % GW == 0
    ngrp = n_w // GW
    nc = Bacc("TRN2", target_bir_lowering=False)

    x_d = nc.dram_tensor("x", (n_w, N, DIM), F32, kind="ExternalInput")
    wqk_d = nc.dram_tensor("wqk", (4, 128, 2 * DIM), F16, kind="ExternalInput")
    wv_d = nc.dram_tensor("wv", (4, 128, DIM), F16, kind="ExternalInput")
    wp_d = nc.dram_tensor("wp", (4, 128, DIM), F16, kind="ExternalInput")
    bqk_d = nc.dram_tensor("bqk", (128, 8), F32, kind="ExternalInput")
    bv_d = nc.dram_tensor("bv", (DIM,), F32, kind="ExternalInput")
    bp_d = nc.dram_tensor("bp", (DIM,), F32, kind="ExternalInput")
    tab_d = nc.dram_tensor("tab", (RPE, 128), F16, kind="ExternalInput")
    idx_d = nc.dram_tensor("idx", (128, _GQ * 8 * _NGATHER), I16,
                           kind="ExternalInput")
    mb_d = nc.dram_tensor("mb", (NKC, n_w * 2), F32, kind="ExternalInput")
    ident_d = nc.dram_tensor("ident", (128, 128), F16, kind="ExternalInput")
    out_d = nc.dram_tensor("out", (n_w, N, DIM), F32, kind="ExternalOutput")

    x16_d = nc.dram_tensor("x16", (n_w * N, DIM), F16)

    with tile.TileContext(nc) as tc, contextlib.ExitStack() as ctx:
        const = ctx.enter_context(tc.tile_pool(name="const", bufs=1))
        gpool = ctx.enter_context(tc.tile_pool(name="gather", bufs=2))
        xt_pool = ctx.enter_context(tc.tile_pool(name="xt", bufs=2))
        qk_pool = ctx.enter_context(tc.tile_pool(name="qk", bufs=2))
        v_pool = ctx.enter_context(tc.tile_pool(name="v", bufs=2))
        praw_pool = ctx.enter_context(tc.tile_pool(name="praw", bufs=3))
        p_pool = ctx.enter_context(tc.tile_pool(name="p", bufs=4))
        o_pool = ctx.enter_context(tc.tile_pool(name="o", bufs=2))
        y_pool = ctx.enter_context(tc.tile_pool(name="y", bufs=3))
        rz_pool = ctx.enter_context(tc.tile_pool(name="rz", bufs=4))
        ps_s = ctx.enter_context(tc.tile_pool(name="ps_s", bufs=1, space="PSUM"))
        ps_a = ctx.enter_context(tc.tile_pool(name="ps_a", bufs=4, space="PSUM"))

        # ---------------- constants ----------------
        wqk_sb = const.tile([128, 4, 2 * DIM], F16)   # [c128, ci, o]  (q|k)
        wv_sb = const.tile([128, 4, DIM], F16)
        wp_sb = const.tile([128, 4, DIM], F16)
        nc.sync.dma_start(out=wqk_sb, in_=wqk_d[:].rearrange("a b c -> b a c"))
        nc.sync.dma_start(out=wv_sb, in_=wv_d[:].rearrange("a b c -> b a c"))
        nc.sync.dma_start(out=wp_sb, in_=wp_d[:].rearrange("a b c -> b a c"))
        ident_sb = const.tile([128, 128], F16)
        nc.sync.dma_start(out=ident_sb, in_=ident_d[:])
        ones_sb = const.tile([NKC, HD], F16)
        nc.vector.memset(ones_sb, 1.0)
        mb_sb = const.tile([NKC, n_w * 2], F32)
        nc.sync.dma_start(out=mb_sb, in_=mb_d[:])
        bqk_sb = const.tile([128, 8], F32)            # per-partition qk bias
        nc.sync.dma_start(out=bqk_sb, in_=bqk_d[:])
        bv_bc = const.tile([128, DIM], F32)           # broadcast rows
        nc.sync.dma_start(
            out=bv_bc, in_=bass.AP(tensor=bv_d[:].tensor, offset=0,
                                   ap=[[0, 128], [1, DIM]]))
        bp_bc = const.tile([128, DIM], F32)
        nc.sync.dma_start(
            out=bp_bc, in_=bass.AP(tensor=bp_d[:].tensor, offset=0,
                                   ap=[[0, 128], [1, DIM]]))
        idx_sb = const.tile([128, _GQ * 8 * _NGATHER], I16)
        nc.sync.dma_start(out=idx_sb, in_=idx_d[:])

        # gathered exp(rpe bias): eb2[p, h, cq] = exp_tab[idx[cq*128+p], h]
        eb2 = const.tile([128, H, 2 * N], F16)
        for g in range(_NGATHER):
            g_sb = gpool.tile([128, _GQ, 128], F16, tag="gather")
            n_idx = _GQ * 128
            nc.gpsimd.dma_gather(
                out_ap=g_sb[:],
                in_ap=tab_d[:],
                idxs_ap=idx_sb[:, g * _GQ * 8:(g + 1) * _GQ * 8],
                num_idxs=n_idx,
                num_idxs_reg=n_idx,
                elem_size=128,
                single_packet=False,
            )
            nc.vector.tensor_copy(
                out=eb2[:, :, g * _GQ:(g + 1) * _GQ],
                in_=g_sb[:, :, 0:H].rearrange("p q h -> p h q"),
            )

        for rep in range(n_rep):
            # ---------------- main loop over 4-window groups ----------------
            for g in range(ngrp):
                tok0 = g * GW * N
                nc.gpsimd.dma_start(
                    out=x16_d[tok0:tok0 + GW * N, :],
                    in_=x_d[:].rearrange(
                        "w n c -> (w n) c")[tok0:tok0 + GW * N, :],
                )
                xt = xt_pool.tile([128, 4, GW * N], F16, tag="xt")
                for ci in range(4):
                    nc.sync.dma_start_transpose(
                        out=xt[:, ci, :],
                        in_=x16_d[tok0:tok0 + GW * N, ci * 128:(ci + 1) * 128],
                    )

                # Q^T / K^T  [o-chunk 128, tok] fp16
                qk_sb = qk_pool.tile([128, 8, GW * N], F16, tag="qk")
                for oc in range(8):
                    for half in range(2):
                        mm_ps = ps_a.tile([128, 512], F32, tag="ps_a")
                        for ci in range(4):
                            nc.tensor.matmul(
                                mm_ps[:, 0:392],
                                lhsT=wqk_sb[:, ci, oc * 128:(oc + 1) * 128],
                                rhs=xt[:, ci, half * 392:(half + 1) * 392],
                                start=(ci == 0), stop=(ci == 3),
                            )
                        nc.any.tensor_scalar_add(
                            out=qk_sb[:, oc, half * 392:(half + 1) * 392],
                            in0=mm_ps[:, 0:392],
                            scalar1=bqk_sb[:, oc:oc + 1],
                        )

                # V natural  [tok-chunk 98, 512] fp16
                v_sb = v_pool.tile([NKC, GW, 2, DIM], F16, tag="v")
                for wi in range(GW):
                    for tcn in range(2):
                        vv_ps = ps_a.tile([128, 512], F32, tag="ps_a")
                        for ci in range(4):
                            nc.tensor.matmul(
                                vv_ps[0:NKC, :],
                                lhsT=xt[:, ci, wi * N + tcn * NKC:
                                        wi * N + (tcn + 1) * NKC],
                                rhs=wv_sb[:, ci, :],
                                start=(ci == 0), stop=(ci == 3),
                            )
                        nc.vector.tensor_add(
                            out=v_sb[:, wi, tcn, :],
                            in0=vv_ps[0:NKC, :],
                            in1=bv_bc[0:NKC, :],
                        )

                # ---------------- attention per window ----------------
                for wi in range(GW):
                    w_abs = g * GW + wi
                    oT = o_pool.tile([128, 4, N], F16, tag="oT")
                    for hg in range(4):
                        p_raw = praw_pool.tile([NKC, 2, 4, N], F16, tag="praw")
                        p_sb = p_pool.tile([NKC, 2, 4, N], F16, tag="p")
                        s_ps = ps_s.tile([128, 4, 512], F32, tag="s")
                        for c in range(2):
                            for i in range(4):           # head = 4*hg + i
                                nc.tensor.matmul(
                                    s_ps[0:NKC, i, c * 196:c * 196 + 196],
                                    lhsT=qk_sb[32 * i:32 * (i + 1), 4 + hg,
                                               wi * N + c * NKC:
                                               wi * N + (c + 1) * NKC],
                                    rhs=qk_sb[32 * i:32 * (i + 1), hg,
                                              wi * N:(wi + 1) * N],
                                    start=True, stop=True,
                                    tile_position=(32 * i, 0),
                                )
                            nc.scalar.activation(
                                out=p_raw[:, c, :, :],
                                in_=s_ps[0:NKC, :, c * 196:c * 196 + 196],
                                func=mybir.ActivationFunctionType.Exp,
                                bias=mb_sb[:, 2 * w_abs + c:2 * w_abs + c + 1],
                                scale=1.0,
                            )
                            nc.vector.tensor_mul(
                                out=p_sb[:, c, :, :],
                                in0=p_raw[:, c, :, :],
                                in1=eb2[0:NKC, 4 * hg:4 * hg + 4,
                                        c * N:(c + 1) * N],
                            )
                        # PV + Z, col-packed over the 4 heads
                        o_ps = ps_a.tile([128, 512], F32, tag="ps_a")
                        z_ps = ps_a.tile([128, 512], F32, tag="ps_a")
                        for i in range(4):
                            h = 4 * hg + i
                            for c in range(2):
                                nc.tensor.matmul(
                                    o_ps[32 * i:32 * (i + 1), 0:N],
                                    lhsT=v_sb[:, wi, c, 32 * h:32 * (h + 1)],
                                    rhs=p_sb[:, c, i, :],
                                    start=(c == 0), stop=(c == 1),
                                    tile_position=(0, 32 * i),
                                )
                            for c in range(2):
                                nc.tensor.matmul(
                                    z_ps[32 * i:32 * (i + 1), 0:N],
                                    lhsT=ones_sb[:],
                                    rhs=p_sb[:, c, i, :],
                                    start=(c == 0), stop=(c == 1),
                                    tile_position=(0, 32 * i),
                                )
                        rz = rz_pool.tile([128, N], F32, tag="rz")
                        nc.vector.reciprocal(out=rz[:], in_=z_ps[:, 0:N])
                        nc.vector.tensor_mul(
                            out=oT[:, hg, :], in0=o_ps[:, 0:N], in1=rz[:])

                    # ---------------- proj ----------------
                    for qc in range(2):
                        y_ps = ps_a.tile([128, 512], F32, tag="ps_a")
                        for hg in range(4):
                            nc.tensor.matmul(
                                y_ps[0:NKC, :],
                                lhsT=oT[:, hg, qc * NKC:(qc + 1) * NKC],
                                rhs=wp_sb[:, hg, :],
                                start=(hg == 0), stop=(hg == 3),
                            )
                        y_sb = y_pool.tile([NKC, DIM], F32, tag="y")
                        nc.vector.tensor_add(
                            out=y_sb[:], in0=y_ps[0:NKC, :],
                            in1=bp_bc[0:NKC, :])
                        nc.sync.dma_start(
                            out=out_d[w_abs, qc * NKC:(qc + 1) * NKC, :],
                            in_=y_sb[:],
                        )
    nc.compile()
    return nc


def _build_v3(n_w=W, n_rep=1, tr_f32=False, no_bcast=False, stage=6, sub=0):
    """v3: flipped PV with ones-augmented V — one matmul pass computes both
    O (natural, q-partitioned) and the softmax denominator Z (33rd column),
    normalize is a per-partition-scalar multiply, O is PE-transposed for
    proj. RPE bias applied as exp-table multiply on DVE (as v2).
    tr_f32: transpose O in f32 instead of fp16 (fp16 PSUM suspect).
    no_bcast: normalize via per-head tensor_scalar instead of stride-0."""
    assert n_w % GW == 0
    ngrp = n_w // GW
    nc = Bacc("TRN2", target_bir_lowering=False)

    x_d = nc.dram_tensor("x", (n_w, N, DIM), F32, kind="ExternalInput")
    wqk_d = nc.dram_tensor("wqk", (4, 128, 2 * DIM), F16, kind="ExternalInput")
    wv_d = nc.dram_tensor("wv", (4, 128, DIM), F16, kind="ExternalInput")
    wp_d = nc.dram_tensor("wp", (4, 128, DIM), F16, kind="ExternalInput")
    bqk_d = nc.dram_tensor("bqk", (128, 8), F32, kind="ExternalInput")
    bv_d = nc.dram_tensor("bv", (DIM,), F32, kind="ExternalInput")
    bp_d = nc.dram_tensor("bp", (DIM,), F32, kind="ExternalInput")
    ebh_d = nc.dram_tensor("ebh", (128, H, 2 * N), F16, kind="ExternalInput")
    mb_d = nc.dram_tensor("mb", (NKC, n_w * 2), F32, kind="ExternalInput")
    ident_d = nc.dram_tensor("ident", (128, 128), F16, kind="ExternalInput")
    out_d = nc.dram_tensor("out", (n_w, N, DIM), F32, kind="ExternalOutput")

    x16_d = nc.dram_tensor("x16", (n_w * N, DIM), F16)

    with tile.TileContext(nc) as tc, contextlib.ExitStack() as ctx:
        const = ctx.enter_context(tc.tile_pool(name="const", bufs=1))
        gpool = ctx.enter_context(tc.tile_pool(name="gather", bufs=2))
        xt_pool = ctx.enter_context(tc.tile_pool(name="xt", bufs=2))
        qk_pool = ctx.enter_context(tc.tile_pool(name="qk", bufs=2))
        v_pool = ctx.enter_context(tc.tile_pool(name="v", bufs=2))
        praw_pool = ctx.enter_context(tc.tile_pool(name="praw", bufs=4))
        p_pool = ctx.enter_context(tc.tile_pool(name="p", bufs=6))
        o_pool = ctx.enter_context(tc.tile_pool(name="o", bufs=2))
        ot_pool = ctx.enter_context(tc.tile_pool(name="ot", bufs=2))
        rz_pool = ctx.enter_context(tc.tile_pool(name="rz", bufs=4))
        y_pool = ctx.enter_context(tc.tile_pool(name="y", bufs=3))
        ps_s = ctx.enter_context(tc.tile_pool(name="ps_s", bufs=1,
                                              space="PSUM"))
        ps_og = ctx.enter_context(tc.tile_pool(name="ps_og", bufs=2,
                                               space="PSUM"))
        ps_a = ctx.enter_context(tc.tile_pool(name="ps_a", bufs=2,
                                              space="PSUM"))

        # ---------------- constants ----------------
        wqk_sb = const.tile([128, 4, 2 * DIM], F16)   # [c128, ci, o]  (q|k)
        wv_sb = const.tile([128, 4, DIM], F16)
        wp_sb = const.tile([128, 4, DIM], F16)
        nc.sync.dma_start(out=wqk_sb, in_=wqk_d[:].rearrange("a b c -> b a c"))
        nc.sync.dma_start(out=wv_sb, in_=wv_d[:].rearrange("a b c -> b a c"))
        nc.sync.dma_start(out=wp_sb, in_=wp_d[:].rearrange("a b c -> b a c"))
        ident_sb = const.tile([128, 128], F16)
        nc.sync.dma_start(out=ident_sb, in_=ident_d[:])
        if tr_f32:
            ident32_sb = const.tile([128, 128], F32)
            nc.vector.tensor_copy(out=ident32_sb, in_=ident_sb[:])
        mb_sb = const.tile([NKC, n_w * 2], F32)
        nc.sync.dma_start(out=mb_sb, in_=mb_d[:])
        bqk_sb = const.tile([128, 8], F32)            # per-partition qk bias
        nc.sync.dma_start(out=bqk_sb, in_=bqk_d[:])
        bv_bc = const.tile([128, DIM], F32)           # broadcast rows
        nc.sync.dma_start(
            out=bv_bc, in_=bass.AP(tensor=bv_d[:].tensor, offset=0,
                                   ap=[[0, 128], [1, DIM]]))
        bp_bc = const.tile([128, DIM], F32)
        nc.sync.dma_start(
            out=bp_bc, in_=bass.AP(tensor=bp_d[:].tensor, offset=0,
                                   ap=[[0, 128], [1, DIM]]))
        # host-gathered exp(rpe bias): eb2[p, h, cq] = exp_tab[idx[cq*128+p], h]
        eb2 = const.tile([128, H, 2 * N], F16)
        nc.sync.dma_start(out=eb2, in_=ebh_d[:])

        for rep in range(n_rep):
            # ------------- main loop over 4-window groups -------------
            for g in range(ngrp):
                tok0 = g * GW * N
                nc.gpsimd.dma_start(
                    out=x16_d[tok0:tok0 + GW * N, :],
                    in_=x_d[:].rearrange(
                        "w n c -> (w n) c")[tok0:tok0 + GW * N, :],
                )
                xt = xt_pool.tile([128, 4, GW * N], F16, tag="xt")
                for ci in range(4):
                    nc.sync.dma_start_transpose(
                        out=xt[:, ci, :],
                        in_=x16_d[tok0:tok0 + GW * N, ci * 128:(ci + 1) * 128],
                    )

                # Q^T / K^T  [o-chunk 128, tok] fp16
                qk_sb = qk_pool.tile([128, 8, GW * N], F16, tag="qk")
                for oc in range(8):
                    for half in range(2):
                        mm_ps = ps_a.tile([128, 512], F32, tag="ps_a")
                        for ci in range(4):
                            nc.tensor.matmul(
                                mm_ps[:, 0:392],
                                lhsT=wqk_sb[:, ci, oc * 128:(oc + 1) * 128],
                                rhs=xt[:, ci, half * 392:(half + 1) * 392],
                                start=(ci == 0), stop=(ci == 3),
                            )
                        nc.any.tensor_scalar_add(
                            out=qk_sb[:, oc, half * 392:(half + 1) * 392],
                            in0=mm_ps[:, 0:392],
                            scalar1=bqk_sb[:, oc:oc + 1],
                        )

                # V natural  [tok-chunk 98, h, 33] fp16 (col 32 = ones)
                v_sb = v_pool.tile([NKC, GW, 2, H, 33], F16, tag="v")
                nc.vector.memset(v_sb[:, :, :, :, 32:33], 1.0)
                for wi in range(GW):
                    for tcn in range(2):
                        vv_ps = ps_a.tile([128, 512], F32, tag="ps_a")
                        for ci in range(4):
                            nc.tensor.matmul(
                                vv_ps[0:NKC, :],
                                lhsT=xt[:, ci, wi * N + tcn * NKC:
                                        wi * N + (tcn + 1) * NKC],
                                rhs=wv_sb[:, ci, :],
                                start=(ci == 0), stop=(ci == 3),
                            )
                        nc.vector.tensor_add(
                            out=v_sb[:, wi, tcn, :, 0:32],
                            in0=vv_ps[0:NKC, :].rearrange(
                                "p (h d) -> p h d", h=H),
                            in1=bv_bc[0:NKC, :].rearrange(
                                "p (h d) -> p h d", h=H),
                        )

                # ---------------- attention per window ----------------
                for wi in range(GW):
                    w_abs = g * GW + wi
                    if stage < 2:
                        for qc in range(2):
                            y_sb = y_pool.tile([NKC, DIM], F32, tag="y")
                            nc.vector.memset(y_sb[:], 0.0)
                            nc.sync.dma_start(
                                out=out_d[w_abs, qc * NKC:(qc + 1) * NKC, :],
                                in_=y_sb[:])
                        continue
                    p_tiles = []
                    for hg in range(4):
                        p_raw = praw_pool.tile([NKC, 2, 4, N], F16,
                                               tag="praw")
                        p_sb = p_pool.tile([NKC, 2, 4, N], F16, tag="p")
                        p_tiles.append(p_sb)
                        s_ps = ps_s.tile([128, 4, 512], F32, tag="s")
                        for c in range(2):
                            for i in range(4):        # head = 4*hg + i
                                nc.tensor.matmul(
                                    s_ps[0:NKC, i, c * 196:c * 196 + 196],
                                    lhsT=qk_sb[32 * i:32 * (i + 1), 4 + hg,
                                               wi * N + c * NKC:
                                               wi * N + (c + 1) * NKC],
                                    rhs=qk_sb[32 * i:32 * (i + 1), hg,
                                              wi * N:(wi + 1) * N],
                                    start=True, stop=True,
                                    tile_position=(32 * i, 0),
                                )
                            nc.scalar.activation(
                                out=p_raw[:, c, :, :],
                                in_=s_ps[0:NKC, :, c * 196:c * 196 + 196],
                                func=mybir.ActivationFunctionType.Exp,
                                bias=mb_sb[:, 2 * w_abs + c:
                                           2 * w_abs + c + 1],
                                scale=1.0,
                            )
                            eb_eng = nc.gpsimd if hg % 2 else nc.vector
                            eb_eng.tensor_mul(
                                out=p_sb[:, c, :, :],
                                in0=p_raw[:, c, :, :],
                                in1=eb2[0:NKC, 4 * hg:4 * hg + 4,
                                        c * N:(c + 1) * N],
                            )

                    if stage < 3:
                        for qc in range(2):
                            y_sb = y_pool.tile([NKC, DIM], F32, tag="y")
                            nc.vector.memset(y_sb[:], 0.0)
                            nc.sync.dma_start(
                                out=out_d[w_abs, qc * NKC:(qc + 1) * NKC, :],
                                in_=y_sb[:])
                        continue
                    # PV+Z flipped: og[q, (h8, 33)] per (qb, grp)
                    o_sb = o_pool.tile([NKC, 2, DIM], F16, tag="o")
                    for qb in range(2):
                        for grp in range(2):
                            og = ps_og.tile([NKC, 512], F32, tag="og")
                            for h8 in range(8):
                                h = grp * 8 + h8
                                for c in range(2):
                                    nc.tensor.matmul(
                                        og[0:NKC, 33 * h8:33 * h8 + 33],
                                        lhsT=p_tiles[h // 4][
                                            :, c, h % 4,
                                            qb * NKC:(qb + 1) * NKC],
                                        rhs=v_sb[:, wi, c, h, :],
                                        start=(c == 0), stop=(c == 1),
                                    )
                            if stage < 4:
                                y_sb = y_pool.tile([NKC, DIM], F32, tag="y")
                                nc.vector.tensor_copy(
                                    out=y_sb[:, 0:264], in_=og[0:NKC, 0:264])
                                nc.vector.memset(y_sb[:, 264:512], 0.0)
                                nc.sync.dma_start(
                                    out=out_d[w_abs,
                                              qb * NKC:(qb + 1) * NKC, :],
                                    in_=y_sb[:])
                                continue
                            ogv = og[0:NKC, 0:264].rearrange(
                                "p (h d) -> p h d", d=33)
                            rz = rz_pool.tile([NKC, 8], F32, tag="rz")
                            nc.vector.reciprocal(out=rz[:], in_=ogv[:, :, 32])
                            if no_bcast:
                                for h8 in range(8):
                                    nc.vector.tensor_scalar_mul(
                                        out=o_sb[:, qb,
                                                 grp * 256 + 32 * h8:
                                                 grp * 256 + 32 * h8 + 32],
                                        in0=ogv[:, h8, 0:32],
                                        scalar1=rz[:, h8:h8 + 1],
                                    )
                            else:
                                nc.vector.tensor_mul(
                                    out=o_sb[:, qb, grp * 256:grp * 256 + 256
                                             ].rearrange(
                                                 "p (h d) -> p h d", h=8),
                                    in0=ogv[:, :, 0:32],
                                    in1=rz[:].broadcast_to((NKC, 8, 32)),
                                )

                    if stage < 4:
                        continue
                    if stage < 5:
                        for qc in range(2):
                            y_sb = y_pool.tile([NKC, DIM], F32, tag="y")
                            nc.vector.tensor_copy(
                                out=y_sb[:], in_=o_sb[:, qc, :])
                            nc.sync.dma_start(
                                out=out_d[w_abs, qc * NKC:(qc + 1) * NKC, :],
                                in_=y_sb[:])
                        continue
                    # transpose O -> O^T and proj
                    ot_sb = ot_pool.tile([128, 4, 2, NKC], F16, tag="ot")
                    if tr_f32:
                        o32_sb = o_pool.tile([NKC, 2, DIM], F32, tag="o32")
                        nc.scalar.activation(
                            out=o32_sb[:], in_=o_sb[:],
                            func=mybir.ActivationFunctionType.Copy)
                    for cc in range(4):
                        for qb in range(2):
                            if tr_f32:
                                ot_ps = ps_a.tile([128, 512], F32,
                                                  tag="ps_a")
                                nc.tensor.transpose(
                                    out=ot_ps[:, 0:NKC],
                                    in_=o32_sb[:, qb,
                                               cc * 128:(cc + 1) * 128],
                                    identity=ident32_sb[0:NKC, 0:NKC],
                                )
                            else:
                                ot_ps = ps_a.tile([128, 1024], F16,
                                                  tag="ps_a")
                                nc.tensor.transpose(
                                    out=ot_ps[:, 0:NKC],
                                    in_=o_sb[:, qb, cc * 128:(cc + 1) * 128],
                                    identity=ident_sb[0:NKC, 0:NKC],
                                )
                            nc.vector.tensor_copy(
                                out=ot_sb[:, cc, qb, :],
                                in_=ot_ps[:, 0:NKC],
                            )
                    if stage < 6:
                        for qc in range(2):
                            y_sb = y_pool.tile([NKC, DIM], F32, tag="y")
                            nc.vector.tensor_copy(
                                out=y_sb[:],
                                in_=ot_sb[:, :, qc, :].rearrange(
                                    "p a b -> p (a b)")[0:NKC, 0:DIM])
                            nc.sync.dma_start(
                                out=out_d[w_abs, qc * NKC:(qc + 1) * NKC, :],
                                in_=y_sb[:])
                        continue
                    for qc in range(2):
                        y_ps = ps_a.tile([128, 512], F32, tag="ps_a")
                        for cc in range(4):
                            nc.tensor.matmul(
                                y_ps[0:NKC, :],
                                lhsT=ot_sb[:, cc, qc, :],
                                rhs=wp_sb[:, cc, :],
                                start=(cc == 0), stop=(cc == 3),
                            )
                        y_sb = y_pool.tile([NKC, DIM], F32, tag="y")
                        nc.vector.tensor_add(
                            out=y_sb[:], in0=y_ps[0:NKC, :],
                            in1=bp_bc[0:NKC, :])
                        nc.sync.dma_start(
                            out=out_d[w_abs, qc * NKC:(qc + 1) * NKC, :],
                            in_=y_sb[:],
                        )
    nc.compile()
    return nc


def _host_prep(x, rpe_index, mask, qkv_w, qkv_b, proj_w, proj_b, rpe_table,
               n_w=W, n_cores=NCORES, exp_tab=True):
    """Shard + layout/dtype prep (numpy only). Returns per-core input maps."""
    x = np.asarray(x, dtype=np.float32)
    rpe_index = np.asarray(rpe_index).astype(np.int64)
    mask = np.asarray(mask).astype(np.int32)
    qkv_w = np.asarray(qkv_w, dtype=np.float32)
    qkv_b = np.asarray(qkv_b, dtype=np.float32)
    proj_w = np.asarray(proj_w, dtype=np.float32)
    proj_b = np.asarray(proj_b, dtype=np.float32)
    rpe_table = np.asarray(rpe_table, dtype=np.float32)

    scale = HD ** -0.5
    wq = qkv_w[0:DIM] * scale
    wk = qkv_w[DIM:2 * DIM]
    wv = qkv_w[2 * DIM:3 * DIM]
    wqk_t = np.concatenate([wq, wk], axis=0).T.astype(np.float16)  # [c, 1024]
    wv_t = wv.T.astype(np.float16)                                 # [c, 512]
    wp_t = proj_w.T.astype(np.float16)                             # [c, 512]
    wqk_t = np.ascontiguousarray(wqk_t.reshape(4, 128, 2 * DIM))
    wv_t = np.ascontiguousarray(wv_t.reshape(4, 128, DIM))
    wp_t = np.ascontiguousarray(wp_t.reshape(4, 128, DIM))

    bqk = np.concatenate([qkv_b[0:DIM] * scale, qkv_b[DIM:2 * DIM]])
    bqk_pp = np.ascontiguousarray(
        bqk.reshape(8, 128).T.astype(np.float32))                  # [128, 8]
    bv = qkv_b[2 * DIM:3 * DIM].astype(np.float32)

    tab = np.zeros((RPE, 128), dtype=np.float16)
    tab_vals = np.exp(rpe_table) if exp_tab else rpe_table
    tab[:, 0:H] = tab_vals.astype(np.float16)
    tab16 = np.ascontiguousarray(tab[:, 0:H])

    # gather index stream: position j = cq*128 + p ; cq = c*196+q ; k = 98c+p
    cq = np.arange(2 * N)
    c = cq // N
    q = cq % N
    p = np.arange(128)
    k = (NKC * c)[:, None] + p[None, :]                            # [392, 128]
    valid = p[None, :] < NKC
    j_idx = np.where(valid, rpe_index[q[:, None] * N + np.minimum(k, N - 1)], 0)
    j_idx = j_idx.reshape(-1).astype(np.int16)
    n_per = _GQ * 128
    idx_w = np.zeros((16, (n_per // 16) * _NGATHER), dtype=np.int16)
    for gch in range(_NGATHER):
        blk = j_idx[gch * n_per:(gch + 1) * n_per].reshape(n_per // 16, 16).T
        idx_w[:, gch * (n_per // 16):(gch + 1) * (n_per // 16)] = blk
    idx_w = np.ascontiguousarray(np.tile(idx_w, (8, 1)))           # [128, .]

    # host-gathered bias table: ebh[p, h, cq] = tab[j_idx[cq, p], h]
    j2 = j_idx.reshape(2 * N, 128)                                 # [cq, p]
    ebh = np.ascontiguousarray(
        tab[j2.astype(np.int64), 0:H].transpose(1, 2, 0))          # [p, h, cq]

    ident = np.eye(128, dtype=np.float16)

    in_maps = []
    for core in range(n_cores):
        xs = x[core * n_w:(core + 1) * n_w]
        ms = mask[core * n_w:(core + 1) * n_w]
        mbv = np.where(ms.astype(bool), EXP_SHIFT, MASK_NEG).astype(np.float32)
        mb = np.zeros((NKC, n_w * 2), dtype=np.float32)
        for wi in range(n_w):
            for cc in range(2):
                mb[:, 2 * wi + cc] = mbv[wi, cc * NKC:(cc + 1) * NKC]
        in_maps.append({
            "x": np.ascontiguousarray(xs),
            "wqk": wqk_t, "wv": wv_t, "wp": wp_t,
            "bqk": bqk_pp, "bv": bv, "bp": proj_b.astype(np.float32),
            "tab": tab, "ebh": ebh, "idx": idx_w,
            "mb": np.ascontiguousarray(mb),
            "ident": ident,
        })
    return in_maps


_NC_CACHE = {}
_BUILDER = _build_v3


def kernel(x, rpe_index, mask, qkv_w, qkv_b, proj_w, proj_b, rpe_table,
           _trace=False):
    from concourse.bass_utils import run_bass_kernel_spmd
    in_maps = _host_prep(x, rpe_index, mask, qkv_w, qkv_b, proj_w, proj_b,
                         rpe_table)
    if "nc" not in _NC_CACHE:
        _NC_CACHE["nc"] = _BUILDER()
    nc = _NC_CACHE["nc"]
    try:
        res = run_bass_kernel_spmd(nc, in_maps, core_ids=list(range(NCORES)),
                                   trace=_trace)
    except ModuleNotFoundError:
        # axon NTFF profiling hook unavailable in this container
        res = run_bass_kernel_spmd(nc, in_maps, core_ids=list(range(NCORES)),
                                   trace=False)
    kernel.last_results = res
    out = np.concatenate([r["out"] for r in res.results], axis=0)
    return out.reshape(B, N, DIM).astype(np.float32)



# revision 44
# speedup vs baseline: 1.9991x; 1.6586x over previous
"""Trainium2 Bass kernel for nn_Attention_4045859193206 (Swin-style window
attention with relative position bias + key masking).

Contract: kernel(**inputs) takes FULL inputs (B=128 windows), shards the batch
across 8 NeuronCores (16 windows each), runs one SPMD Bass kernel, returns the
FULL (128, 196, 512) float32 output.

Self-contained: hardcodes all shapes; no sibling imports.

Design v3 (per core, W=16 windows) — PE streaming cost is the bound, so all
work that does not need the systolic array is moved off it:
  - x cast fp32->fp16 on device (DRAM->DRAM DMA cast), then DMA-transposed to
    x^T [c, tok] in SBUF (4-window groups: 784 tokens, multiple of 16).
  - QKV: Q^T/K^T in transposed form ([o,tok], fp16, q pre-scaled via host-
    scaled weights); V in natural form [tok, h, 33] with a 33rd ones column
    per head appended.
  - S^T = K^T-lhsT matmuls, 4 heads row-packed via tile_position, one PSUM
    bank per PE tile position (matmuls at different tile positions must not
    share a PSUM bank — HW constraint found the hard way).
  - softmax numerator: P = exp(S + mask_bias - 4) on ScalarE (mask as
    per-partition bias AP; -4 cancels in normalization), then multiplied by
    the host-gathered exp(RPE bias) table (eb2 [k, h, q] fp16) on DVE/Pool
    (split by head group) — no PE cycles for the bias at all.
  - PV+Z in ONE flipped matmul pass: out[q-band, (h,33)] with lhsT = P chunk
    [k, q-band], rhs = [V_h | 1] [k, 33].  Column 32 of each head IS the
    softmax denominator Z, and the output lands q-partitioned so the
    normalization is a per-partition reciprocal + broadcast multiply
    (stride-0 free-dim AP), writing natural-layout O fp16.
  - O is PE-transposed (8 x [98,128] fp16 transposes, ~784 PE cycles/window)
    back to O^T for the projection; proj bias added during the final
    PSUM->SBUF pass, DMA out.
  - exp(bias) gather is done on the HOST (rpe_table/rpe_index are host
    visible): eb2 = exp(rpe_table)[rpe_index] shipped as a 1.6 MB input,
    replacing a 12.8 MB on-device dma_gather.
PE streaming per window: S 6272 + PV/Z 2112 + transpose 784 + QK 6272 +
V 4096 + proj 4096 ~= 23.6k cycles; model PE busy 160 us/core, measured
~115-130 us on hardware (baseline 220 us).
"""

import contextlib
import numpy as np

import concourse.bass as bass
import concourse.mybir as mybir
import concourse.tile as tile
from concourse.bacc import Bacc

# ---------------------------------------------------------------- constants
B, N, DIM, H = 128, 196, 512, 16
HD = DIM // H                     # 32
RPE = 729                         # (2*14-1)^2
NCORES = 8
W = B // NCORES                   # 16 windows per core
NKC = 98                          # k-chunk (2 chunks of 98 = 196)
GW = 4                            # windows per qkv group (4*196=784 tokens)
F16 = mybir.dt.float16
F32 = mybir.dt.float32
I16 = mybir.dt.int16
EXP_SHIFT = -4.0                  # exp(s-4): fp16 headroom; cancels in softmax
MASK_NEG = -1e9
_GQ = 98                          # (c,q) positions per gather chunk
_NGATHER = 4                      # 4 chunks of 98 positions = 392


def _build_nc(n_w=W, ablate=frozenset(), variant="base"):
    """Build the per-core Bass program for n_w windows.
    ablate: subset of {'z','bias','qk','pv'} - drop those matmuls (timing expts).
    variant: 'base' or 'bundle2' (2-head bias bundling + s_ps 2 banks x 2 bufs)."""
    assert n_w % GW == 0
    ngrp = n_w // GW
    nc = Bacc("TRN2", target_bir_lowering=False)

    x_d = nc.dram_tensor("x", (n_w, N, DIM), F32, kind="ExternalInput")
    wqk_d = nc.dram_tensor("wqk", (4, 128, 2 * DIM), F16, kind="ExternalInput")
    wv_d = nc.dram_tensor("wv", (4, 128, DIM), F16, kind="ExternalInput")
    wp_d = nc.dram_tensor("wp", (4, 128, DIM), F16, kind="ExternalInput")
    bqk_d = nc.dram_tensor("bqk", (128, 8), F32, kind="ExternalInput")
    bv_d = nc.dram_tensor("bv", (DIM,), F32, kind="ExternalInput")
    bp_d = nc.dram_tensor("bp", (DIM,), F32, kind="ExternalInput")
    tab_d = nc.dram_tensor("tab", (RPE, 128), F16, kind="ExternalInput")
    idx_d = nc.dram_tensor("idx", (128, _GQ * 8 * _NGATHER), I16,
                           kind="ExternalInput")
    mb_d = nc.dram_tensor("mb", (NKC, n_w * 2), F32, kind="ExternalInput")
    ident_d = nc.dram_tensor("ident", (128, 128), F16, kind="ExternalInput")
    out_d = nc.dram_tensor("out", (n_w, N, DIM), F32, kind="ExternalOutput")

    x16_d = nc.dram_tensor("x16", (n_w * N, DIM), F16)

    with tile.TileContext(nc) as tc, contextlib.ExitStack() as ctx:
        const = ctx.enter_context(tc.tile_pool(name="const", bufs=1))
        gpool = ctx.enter_context(tc.tile_pool(name="gather", bufs=2))
        xt_pool = ctx.enter_context(tc.tile_pool(
            name="xt", bufs=(3 if variant == "tune2" else 2)))
        qk_pool = ctx.enter_context(tc.tile_pool(
            name="qk", bufs=(3 if variant == "tune2" else 2)))
        v_pool = ctx.enter_context(tc.tile_pool(name="v", bufs=2))
        p_pool = ctx.enter_context(tc.tile_pool(
            name="p", bufs=(4 if variant in ("tune1", "tune2") else 3)))
        o_pool = ctx.enter_context(tc.tile_pool(
            name="o", bufs=(3 if variant == "tune2" else 2)))
        y_pool = ctx.enter_context(tc.tile_pool(name="y", bufs=3))
        rz_pool = ctx.enter_context(tc.tile_pool(
            name="rz", bufs=(4 if variant == "tune2" else 3)))
        ps_s = ctx.enter_context(tc.tile_pool(
            name="ps_s", bufs=(2 if variant == "bundle2" else 1), space="PSUM"))
        ps_a = ctx.enter_context(tc.tile_pool(name="ps_a", bufs=4, space="PSUM"))

        # ---------------- constants ----------------
        wqk_sb = const.tile([128, 4, 2 * DIM], F16)   # [c128, ci, o]  (q|k)
        wv_sb = const.tile([128, 4, DIM], F16)
        wp_sb = const.tile([128, 4, DIM], F16)
        nc.sync.dma_start(out=wqk_sb, in_=wqk_d[:].rearrange("a b c -> b a c"))
        nc.sync.dma_start(out=wv_sb, in_=wv_d[:].rearrange("a b c -> b a c"))
        nc.sync.dma_start(out=wp_sb, in_=wp_d[:].rearrange("a b c -> b a c"))
        ident_sb = const.tile([128, 128], F16)
        nc.sync.dma_start(out=ident_sb, in_=ident_d[:])
        ones_sb = const.tile([NKC, HD], F16)
        nc.vector.memset(ones_sb, 1.0)
        mb_sb = const.tile([NKC, n_w * 2], F32)
        nc.sync.dma_start(out=mb_sb, in_=mb_d[:])
        bqk_sb = const.tile([128, 8], F32)            # per-partition qk bias
        nc.sync.dma_start(out=bqk_sb, in_=bqk_d[:])
        bv_bc = const.tile([128, DIM], F32)           # broadcast rows
        nc.sync.dma_start(
            out=bv_bc, in_=bass.AP(tensor=bv_d[:].tensor, offset=0,
                                   ap=[[0, 128], [1, DIM]]))
        bp_bc = const.tile([128, DIM], F32)
        nc.sync.dma_start(
            out=bp_bc, in_=bass.AP(tensor=bp_d[:].tensor, offset=0,
                                   ap=[[0, 128], [1, DIM]]))
        idx_sb = const.tile([128, _GQ * 8 * _NGATHER], I16)
        nc.sync.dma_start(out=idx_sb, in_=idx_d[:])

        # gathered rpe bias: bias_sb[p, cq, h] = tab[idx[cq*128+p], h]
        bias_sb = const.tile([128, 2 * N, H], F16)
        for g in range(_NGATHER):
            g_sb = gpool.tile([128, _GQ, 128], F16, tag="gather")
            n_idx = _GQ * 128
            nc.gpsimd.dma_gather(
                out_ap=g_sb[:],
                in_ap=tab_d[:],
                idxs_ap=idx_sb[:, g * _GQ * 8:(g + 1) * _GQ * 8],
                num_idxs=n_idx,
                num_idxs_reg=n_idx,
                elem_size=128,
                single_packet=False,
            )
            nc.vector.tensor_copy(
                out=bias_sb[:, g * _GQ:(g + 1) * _GQ, :],
                in_=g_sb[:, :, 0:H],
            )

        # ---------------- main loop over 4-window groups ----------------
        for g in range(ngrp):
            tok0 = g * GW * N
            nc.gpsimd.dma_start(
                out=x16_d[tok0:tok0 + GW * N, :],
                in_=x_d[:].rearrange("w n c -> (w n) c")[tok0:tok0 + GW * N, :],
            )
            xt = xt_pool.tile([128, 4, GW * N], F16, tag="xt")
            for ci in range(4):
                nc.sync.dma_start_transpose(
                    out=xt[:, ci, :],
                    in_=x16_d[tok0:tok0 + GW * N, ci * 128:(ci + 1) * 128],
                )

            # Q^T / K^T  [o-chunk 128, tok] fp16
            qk_sb = qk_pool.tile([128, 8, GW * N], F16, tag="qk")
            for oc in range(8):
                for half in range(2):
                    mm_ps = ps_a.tile([128, 512], F32, tag="ps_a")
                    for ci in range(4):
                        nc.tensor.matmul(
                            mm_ps[:, 0:392],
                            lhsT=wqk_sb[:, ci, oc * 128:(oc + 1) * 128],
                            rhs=xt[:, ci, half * 392:(half + 1) * 392],
                            start=(ci == 0), stop=(ci == 3),
                        )
                    nc.any.tensor_scalar_add(
                        out=qk_sb[:, oc, half * 392:(half + 1) * 392],
                        in0=mm_ps[:, 0:392],
                        scalar1=bqk_sb[:, oc:oc + 1],
                    )

            # V natural  [tok-chunk 98, 512] fp16
            v_sb = v_pool.tile([NKC, GW, 2, DIM], F16, tag="v")
            for wi in range(GW):
                for tcn in range(2):
                    vv_ps = ps_a.tile([128, 512], F32, tag="ps_a")
                    for ci in range(4):
                        nc.tensor.matmul(
                            vv_ps[0:NKC, :],
                            lhsT=xt[:, ci,
                                    wi * N + tcn * NKC:wi * N + (tcn + 1) * NKC],
                            rhs=wv_sb[:, ci, :],
                            start=(ci == 0), stop=(ci == 3),
                        )
                    nc.vector.tensor_add(
                        out=v_sb[:, wi, tcn, :],
                        in0=vv_ps[0:NKC, :],
                        in1=bv_bc[0:NKC, :],
                    )

            # ---------------- attention per window ----------------
            for wi in range(GW):
                w_abs = g * GW + wi
                oT = o_pool.tile([128, 4, N], F16, tag="oT")
                for hg in range(4):
                    p_sb = p_pool.tile([NKC, 2, 4, N], F16, tag="p")
                    if variant != "bundle2":
                        s_ps = ps_s.tile([128, 4, 512], F32, tag="s")
                    for c in range(2):
                        if variant == "bundle2":
                            s_ps = ps_s.tile([128, 2, 512], F32, tag="s")
                        if "qk" in ablate and "bias" in ablate:
                            nc.tensor.matmul(
                                s_ps[0:NKC, 0, 0:32],
                                lhsT=ident_sb[0:NKC, 0:NKC],
                                rhs=bias_sb[0:NKC, 0:2, 0:16
                                            ].rearrange("p q h -> p (q h)"),
                                start=True, stop=True,
                            )
                        for i in range(4):               # head = 4*hg + i
                            if "qk" in ablate:
                                break
                            if variant == "bundle2":
                                s_out = s_ps[0:NKC, i // 2,
                                             (i % 2) * 196:(i % 2) * 196 + 196]
                                st = (i % 2 == 0)
                            else:
                                s_out = s_ps[0:NKC, i, c * 196:c * 196 + 196]
                                st = True
                            if variant == "tune2":
                                nc.tensor.matmul(
                                    s_ps[0:NKC, i, c * 196:c * 196 + 196],
                                    lhsT=ident_sb[0:NKC, 0:NKC],
                                    rhs=bias_sb[0:NKC, c * N:(c + 1) * N,
                                                4 * hg + i],
                                    start=True, stop=False,
                                )
                            nc.tensor.matmul(
                                s_out,
                                lhsT=qk_sb[32 * i:32 * (i + 1), 4 + hg,
                                           wi * N + c * NKC:
                                           wi * N + (c + 1) * NKC],
                                rhs=qk_sb[32 * i:32 * (i + 1), hg,
                                          wi * N:(wi + 1) * N],
                                start=(st and variant != "tune2"),
                                stop=(variant == "tune2"),
                                tile_position=(32 * i, 0),
                            )
                        if "bias" not in ablate and variant == "bundle2":
                            for pr in range(2):          # head pair
                                nc.tensor.matmul(
                                    s_ps[0:NKC, pr, 0:392],
                                    lhsT=ident_sb[0:NKC, 0:NKC],
                                    rhs=bias_sb[0:NKC, c * N:(c + 1) * N,
                                                4 * hg + 2 * pr:
                                                4 * hg + 2 * pr + 2
                                                ].rearrange("p q h -> p h q"),
                                    start=("qk" in ablate), stop=True,
                                )
                        elif "bias" not in ablate and variant != "tune2":
                            for i in range(4):           # rpe bias, K=98 each
                                h = 4 * hg + i
                                nc.tensor.matmul(
                                    s_ps[0:NKC, i, c * 196:c * 196 + 196],
                                    lhsT=ident_sb[0:NKC, 0:NKC],
                                    rhs=bias_sb[0:NKC, c * N:(c + 1) * N, h],
                                    start=("qk" in ablate), stop=True,
                                )
                        if variant == "batch":
                            continue                     # exps after all MMs
                        if variant == "bundle2":
                            exp_in = s_ps[0:NKC, :, 0:392]
                        else:
                            exp_in = s_ps[0:NKC, :, c * 196:c * 196 + 196]
                        nc.scalar.activation(
                            out=p_sb[:, c, :, :],
                            in_=exp_in,
                            func=mybir.ActivationFunctionType.Exp,
                            bias=mb_sb[:, 2 * w_abs + c:2 * w_abs + c + 1],
                            scale=1.0,
                        )
                    if variant == "batch":
                        for c in range(2):
                            nc.scalar.activation(
                                out=p_sb[:, c, :, :],
                                in_=s_ps[0:NKC, :, c * 196:c * 196 + 196],
                                func=mybir.ActivationFunctionType.Exp,
                                bias=mb_sb[:, 2 * w_abs + c:
                                           2 * w_abs + c + 1],
                                scale=1.0,
                            )
                    # PV + Z, col-packed over the 4 heads
                    o_ps = ps_a.tile([128, 512], F32, tag="ps_a")
                    z_ps = ps_a.tile([128, 512], F32, tag="ps_a")
                    for i in range(4):
                        h = 4 * hg + i
                        if "pv" in ablate and i == 0:
                            nc.tensor.matmul(
                                o_ps[0:32, 0:16],
                                lhsT=v_sb[:, wi, 0, 0:32],
                                rhs=p_sb[:, 0, 0, 0:16],
                                start=True, stop=True,
                            )
                        if "pv" not in ablate:
                            for c in range(2):
                                nc.tensor.matmul(
                                    o_ps[32 * i:32 * (i + 1), 0:N],
                                    lhsT=v_sb[:, wi, c, 32 * h:32 * (h + 1)],
                                    rhs=p_sb[:, c, i, :],
                                    start=(c == 0), stop=(c == 1),
                                    tile_position=(0, 32 * i),
                                )
                        if "z" in ablate and i == 0:
                            nc.tensor.matmul(
                                z_ps[0:32, 0:16],
                                lhsT=ones_sb[:, 0:32],
                                rhs=p_sb[:, 0, 0, 0:16],
                                start=True, stop=True,
                            )
                        if "z" not in ablate:
                            for c in range(2):
                                nc.tensor.matmul(
                                    z_ps[32 * i:32 * (i + 1), 0:N],
                                    lhsT=ones_sb[:],
                                    rhs=p_sb[:, c, i, :],
                                    start=(c == 0), stop=(c == 1),
                                    tile_position=(0, 32 * i),
                                )
                    rz = rz_pool.tile([128, N], F32, tag="rz")
                    if variant in ("tune1", "tune2"):
                        nc.vector.reciprocal(out=rz[:], in_=z_ps[:, 0:N])
                    else:
                        z_sb = rz_pool.tile([128, N], F32, tag="z")
                        nc.scalar.copy(out=z_sb[:], in_=z_ps[:, 0:N])
                        nc.vector.reciprocal_approx_fast(out=rz[:], in_=z_sb[:])
                    nc.vector.tensor_mul(
                        out=oT[:, hg, :], in0=o_ps[:, 0:N], in1=rz[:])

                # ---------------- proj ----------------
                for qc in range(2):
                    y_ps = ps_a.tile([128, 512], F32, tag="ps_a")
                    for hg in range(4):
                        nc.tensor.matmul(
                            y_ps[0:NKC, :],
                            lhsT=oT[:, hg, qc * NKC:(qc + 1) * NKC],
                            rhs=wp_sb[:, hg, :],
                            start=(hg == 0), stop=(hg == 3),
                        )
                    y_sb = y_pool.tile([NKC, DIM], F32, tag="y")
                    nc.vector.tensor_add(
                        out=y_sb[:], in0=y_ps[0:NKC, :], in1=bp_bc[0:NKC, :])
                    nc.sync.dma_start(
                        out=out_d[w_abs, qc * NKC:(qc + 1) * NKC, :],
                        in_=y_sb[:],
                    )
    nc.compile()
    return nc


def _build_v2(n_w=W, n_rep=1):
    """v2: RPE bias applied as exp(bias) multiply on DVE (host pre-exp'd
    table), S matmuls close their own accumulation. n_rep repeats the whole
    compute body inside one program (for slope timing)."""
    assert n_w % GW == 0
    ngrp = n_w // GW
    nc = Bacc("TRN2", target_bir_lowering=False)

    x_d = nc.dram_tensor("x", (n_w, N, DIM), F32, kind="ExternalInput")
    wqk_d = nc.dram_tensor("wqk", (4, 128, 2 * DIM), F16, kind="ExternalInput")
    wv_d = nc.dram_tensor("wv", (4, 128, DIM), F16, kind="ExternalInput")
    wp_d = nc.dram_tensor("wp", (4, 128, DIM), F16, kind="ExternalInput")
    bqk_d = nc.dram_tensor("bqk", (128, 8), F32, kind="ExternalInput")
    bv_d = nc.dram_tensor("bv", (DIM,), F32, kind="ExternalInput")
    bp_d = nc.dram_tensor("bp", (DIM,), F32, kind="ExternalInput")
    tab_d = nc.dram_tensor("tab", (RPE, 128), F16, kind="ExternalInput")
    idx_d = nc.dram_tensor("idx", (128, _GQ * 8 * _NGATHER), I16,
                           kind="ExternalInput")
    mb_d = nc.dram_tensor("mb", (NKC, n_w * 2), F32, kind="ExternalInput")
    ident_d = nc.dram_tensor("ident", (128, 128), F16, kind="ExternalInput")
    out_d = nc.dram_tensor("out", (n_w, N, DIM), F32, kind="ExternalOutput")

    x16_d = nc.dram_tensor("x16", (n_w * N, DIM), F16)

    with tile.TileContext(nc) as tc, contextlib.ExitStack() as ctx:
        const = ctx.enter_context(tc.tile_pool(name="const", bufs=1))
        gpool = ctx.enter_context(tc.tile_pool(name="gather", bufs=2))
        xt_pool = ctx.enter_context(tc.tile_pool(name="xt", bufs=2))
        qk_pool = ctx.enter_context(tc.tile_pool(name="qk", bufs=2))
        v_pool = ctx.enter_context(tc.tile_pool(name="v", bufs=2))
        praw_pool = ctx.enter_context(tc.tile_pool(name="praw", bufs=3))
        p_pool = ctx.enter_context(tc.tile_pool(name="p", bufs=4))
        o_pool = ctx.enter_context(tc.tile_pool(name="o", bufs=2))
        y_pool = ctx.enter_context(tc.tile_pool(name="y", bufs=3))
        rz_pool = ctx.enter_context(tc.tile_pool(name="rz", bufs=4))
        ps_s = ctx.enter_context(tc.tile_pool(name="ps_s", bufs=1, space="PSUM"))
        ps_a = ctx.enter_context(tc.tile_pool(name="ps_a", bufs=4, space="PSUM"))

        # ---------------- constants ----------------
        wqk_sb = const.tile([128, 4, 2 * DIM], F16)   # [c128, ci, o]  (q|k)
        wv_sb = const.tile([128, 4, DIM], F16)
        wp_sb = const.tile([128, 4, DIM], F16)
        nc.sync.dma_start(out=wqk_sb, in_=wqk_d[:].rearrange("a b c -> b a c"))
        nc.sync.dma_start(out=wv_sb, in_=wv_d[:].rearrange("a b c -> b a c"))
        nc.sync.dma_start(out=wp_sb, in_=wp_d[:].rearrange("a b c -> b a c"))
        ident_sb = const.tile([128, 128], F16)
        nc.sync.dma_start(out=ident_sb, in_=ident_d[:])
        ones_sb = const.tile([NKC, HD], F16)
        nc.vector.memset(ones_sb, 1.0)
        mb_sb = const.tile([NKC, n_w * 2], F32)
        nc.sync.dma_start(out=mb_sb, in_=mb_d[:])
        bqk_sb = const.tile([128, 8], F32)            # per-partition qk bias
        nc.sync.dma_start(out=bqk_sb, in_=bqk_d[:])
        bv_bc = const.tile([128, DIM], F32)           # broadcast rows
        nc.sync.dma_start(
            out=bv_bc, in_=bass.AP(tensor=bv_d[:].tensor, offset=0,
                                   ap=[[0, 128], [1, DIM]]))
        bp_bc = const.tile([128, DIM], F32)
        nc.sync.dma_start(
            out=bp_bc, in_=bass.AP(tensor=bp_d[:].tensor, offset=0,
                                   ap=[[0, 128], [1, DIM]]))
        idx_sb = const.tile([128, _GQ * 8 * _NGATHER], I16)
        nc.sync.dma_start(out=idx_sb, in_=idx_d[:])

        # gathered exp(rpe bias): eb2[p, h, cq] = exp_tab[idx[cq*128+p], h]
        eb2 = const.tile([128, H, 2 * N], F16)
        for g in range(_NGATHER):
            g_sb = gpool.tile([128, _GQ, 128], F16, tag="gather")
            n_idx = _GQ * 128
            nc.gpsimd.dma_gather(
                out_ap=g_sb[:],
                in_ap=tab_d[:],
                idxs_ap=idx_sb[:, g * _GQ * 8:(g + 1) * _GQ * 8],
                num_idxs=n_idx,
                num_idxs_reg=n_idx,
                elem_size=128,
                single_packet=False,
            )
            nc.vector.tensor_copy(
                out=eb2[:, :, g * _GQ:(g + 1) * _GQ],
                in_=g_sb[:, :, 0:H].rearrange("p q h -> p h q"),
            )

        for rep in range(n_rep):
            # ---------------- main loop over 4-window groups ----------------
            for g in range(ngrp):
                tok0 = g * GW * N
                nc.gpsimd.dma_start(
                    out=x16_d[tok0:tok0 + GW * N, :],
                    in_=x_d[:].rearrange(
                        "w n c -> (w n) c")[tok0:tok0 + GW * N, :],
                )
                xt = xt_pool.tile([128, 4, GW * N], F16, tag="xt")
                for ci in range(4):
                    nc.sync.dma_start_transpose(
                        out=xt[:, ci, :],
                        in_=x16_d[tok0:tok0 + GW * N, ci * 128:(ci + 1) * 128],
                    )

                # Q^T / K^T  [o-chunk 128, tok] fp16
                qk_sb = qk_pool.tile([128, 8, GW * N], F16, tag="qk")
                for oc in range(8):
                    for half in range(2):
                        mm_ps = ps_a.tile([128, 512], F32, tag="ps_a")
                        for ci in range(4):
                            nc.tensor.matmul(
                                mm_ps[:, 0:392],
                                lhsT=wqk_sb[:, ci, oc * 128:(oc + 1) * 128],
                                rhs=xt[:, ci, half * 392:(half + 1) * 392],
                                start=(ci == 0), stop=(ci == 3),
                            )
                        nc.any.tensor_scalar_add(
                            out=qk_sb[:, oc, half * 392:(half + 1) * 392],
                            in0=mm_ps[:, 0:392],
                            scalar1=bqk_sb[:, oc:oc + 1],
                        )

                # V natural  [tok-chunk 98, 512] fp16
                v_sb = v_pool.tile([NKC, GW, 2, DIM], F16, tag="v")
                for wi in range(GW):
                    for tcn in range(2):
                        vv_ps = ps_a.tile([128, 512], F32, tag="ps_a")
                        for ci in range(4):
                            nc.tensor.matmul(
                                vv_ps[0:NKC, :],
                                lhsT=xt[:, ci, wi * N + tcn * NKC:
                                        wi * N + (tcn + 1) * NKC],
                                rhs=wv_sb[:, ci, :],
                                start=(ci == 0), stop=(ci == 3),
                            )
                        nc.vector.tensor_add(
                            out=v_sb[:, wi, tcn, :],
                            in0=vv_ps[0:NKC, :],
                            in1=bv_bc[0:NKC, :],
                        )

                # ---------------- attention per window ----------------
                for wi in range(GW):
                    w_abs = g * GW + wi
                    oT = o_pool.tile([128, 4, N], F16, tag="oT")
                    for hg in range(4):
                        p_raw = praw_pool.tile([NKC, 2, 4, N], F16, tag="praw")
                        p_sb = p_pool.tile([NKC, 2, 4, N], F16, tag="p")
                        s_ps = ps_s.tile([128, 4, 512], F32, tag="s")
                        for c in range(2):
                            for i in range(4):           # head = 4*hg + i
                                nc.tensor.matmul(
                                    s_ps[0:NKC, i, c * 196:c * 196 + 196],
                                    lhsT=qk_sb[32 * i:32 * (i + 1), 4 + hg,
                                               wi * N + c * NKC:
                                               wi * N + (c + 1) * NKC],
                                    rhs=qk_sb[32 * i:32 * (i + 1), hg,
                                              wi * N:(wi + 1) * N],
                                    start=True, stop=True,
                                    tile_position=(32 * i, 0),
                                )
                            nc.scalar.activation(
                                out=p_raw[:, c, :, :],
                                in_=s_ps[0:NKC, :, c * 196:c * 196 + 196],
                                func=mybir.ActivationFunctionType.Exp,
                                bias=mb_sb[:, 2 * w_abs + c:2 * w_abs + c + 1],
                                scale=1.0,
                            )
                            nc.vector.tensor_mul(
                                out=p_sb[:, c, :, :],
                                in0=p_raw[:, c, :, :],
                                in1=eb2[0:NKC, 4 * hg:4 * hg + 4,
                                        c * N:(c + 1) * N],
                            )
                        # PV + Z, col-packed over the 4 heads
                        o_ps = ps_a.tile([128, 512], F32, tag="ps_a")
                        z_ps = ps_a.tile([128, 512], F32, tag="ps_a")
                        for i in range(4):
                            h = 4 * hg + i
                            for c in range(2):
                                nc.tensor.matmul(
                                    o_ps[32 * i:32 * (i + 1), 0:N],
                                    lhsT=v_sb[:, wi, c, 32 * h:32 * (h + 1)],
                                    rhs=p_sb[:, c, i, :],
                                    start=(c == 0), stop=(c == 1),
                                    tile_position=(0, 32 * i),
                                )
                            for c in range(2):
                                nc.tensor.matmul(
                                    z_ps[32 * i:32 * (i + 1), 0:N],
                                    lhsT=ones_sb[:],
                                    rhs=p_sb[:, c, i, :],
                                    start=(c == 0), stop=(c == 1),
                                    tile_position=(0, 32 * i),
                                )
                        rz = rz_pool.tile([128, N], F32, tag="rz")
                        nc.vector.reciprocal(out=rz[:], in_=z_ps[:, 0:N])
                        nc.vector.tensor_mul(
                            out=oT[:, hg, :], in0=o_ps[:, 0:N], in1=rz[:])

                    # ---------------- proj ----------------
                    for qc in range(2):
                        y_ps = ps_a.tile([128, 512], F32, tag="ps_a")
                        for hg in range(4):
                            nc.tensor.matmul(
                                y_ps[0:NKC, :],
                                lhsT=oT[:, hg, qc * NKC:(qc + 1) * NKC],
                                rhs=wp_sb[:, hg, :],
                                start=(hg == 0), stop=(hg == 3),
                            )
                        y_sb = y_pool.tile([NKC, DIM], F32, tag="y")
                        nc.vector.tensor_add(
                            out=y_sb[:], in0=y_ps[0:NKC, :],
                            in1=bp_bc[0:NKC, :])
                        nc.sync.dma_start(
                            out=out_d[w_abs, qc * NKC:(qc + 1) * NKC, :],
                            in_=y_sb[:],
                        )
    nc.compile()
    return nc


def _build_v3(n_w=W, n_rep=1, tr_f32=False, no_bcast=False, stage=6, sub=0):
    """v3: flipped PV with ones-augmented V — one matmul pass computes both
    O (natural, q-partitioned) and the softmax denominator Z (33rd column),
    normalize is a per-partition-scalar multiply, O is PE-transposed for
    proj. RPE bias applied as exp-table multiply on DVE (as v2).
    tr_f32: transpose O in f32 instead of fp16 (fp16 PSUM suspect).
    no_bcast: normalize via per-head tensor_scalar instead of stride-0."""
    assert n_w % GW == 0
    ngrp = n_w // GW
    nc = Bacc("TRN2", target_bir_lowering=False)

    x_d = nc.dram_tensor("x", (n_w, N, DIM), F32, kind="ExternalInput")
    wqk_d = nc.dram_tensor("wqk", (4, 128, 2 * DIM), F16, kind="ExternalInput")
    wv_d = nc.dram_tensor("wv", (4, 128, DIM), F16, kind="ExternalInput")
    wp_d = nc.dram_tensor("wp", (4, 128, DIM), F16, kind="ExternalInput")
    bqk_d = nc.dram_tensor("bqk", (128, 8), F32, kind="ExternalInput")
    bv_d = nc.dram_tensor("bv", (DIM,), F32, kind="ExternalInput")
    bp_d = nc.dram_tensor("bp", (DIM,), F32, kind="ExternalInput")
    ebh_d = nc.dram_tensor("ebh", (128, H, 2 * N), F16, kind="ExternalInput")
    mb_d = nc.dram_tensor("mb", (NKC, n_w * 2), F32, kind="ExternalInput")
    ident_d = nc.dram_tensor("ident", (128, 128), F16, kind="ExternalInput")
    out_d = nc.dram_tensor("out", (n_w, N, DIM), F32, kind="ExternalOutput")

    x16_d = nc.dram_tensor("x16", (n_w * N, DIM), F16)

    with tile.TileContext(nc) as tc, contextlib.ExitStack() as ctx:
        const = ctx.enter_context(tc.tile_pool(name="const", bufs=1))
        gpool = ctx.enter_context(tc.tile_pool(name="gather", bufs=2))
        xt_pool = ctx.enter_context(tc.tile_pool(name="xt", bufs=2))
        qk_pool = ctx.enter_context(tc.tile_pool(name="qk", bufs=3))
        v_pool = ctx.enter_context(tc.tile_pool(name="v", bufs=3))
        praw_pool = ctx.enter_context(tc.tile_pool(name="praw", bufs=5))
        p_pool = ctx.enter_context(tc.tile_pool(name="p", bufs=8))
        o_pool = ctx.enter_context(tc.tile_pool(name="o", bufs=3))
        ot_pool = ctx.enter_context(tc.tile_pool(name="ot", bufs=3))
        rz_pool = ctx.enter_context(tc.tile_pool(name="rz", bufs=4))
        y_pool = ctx.enter_context(tc.tile_pool(name="y", bufs=3))
        ps_s = ctx.enter_context(tc.tile_pool(name="ps_s", bufs=1,
                                              space="PSUM"))
        ps_og = ctx.enter_context(tc.tile_pool(name="ps_og", bufs=2,
                                               space="PSUM"))
        ps_a = ctx.enter_context(tc.tile_pool(name="ps_a", bufs=2,
                                              space="PSUM"))

        # ---------------- constants ----------------
        wqk_sb = const.tile([128, 4, 2 * DIM], F16)   # [c128, ci, o]  (q|k)
        wv_sb = const.tile([128, 4, DIM], F16)
        wp_sb = const.tile([128, 4, DIM], F16)
        nc.sync.dma_start(out=wqk_sb, in_=wqk_d[:].rearrange("a b c -> b a c"))
        nc.sync.dma_start(out=wv_sb, in_=wv_d[:].rearrange("a b c -> b a c"))
        nc.sync.dma_start(out=wp_sb, in_=wp_d[:].rearrange("a b c -> b a c"))
        ident_sb = const.tile([128, 128], F16)
        nc.sync.dma_start(out=ident_sb, in_=ident_d[:])
        if tr_f32:
            ident32_sb = const.tile([128, 128], F32)
            nc.vector.tensor_copy(out=ident32_sb, in_=ident_sb[:])
        mb_sb = const.tile([NKC, n_w * 2], F32)
        nc.sync.dma_start(out=mb_sb, in_=mb_d[:])
        bqk_sb = const.tile([128, 8], F32)            # per-partition qk bias
        nc.sync.dma_start(out=bqk_sb, in_=bqk_d[:])
        bv_bc = const.tile([128, DIM], F32)           # broadcast rows
        nc.sync.dma_start(
            out=bv_bc, in_=bass.AP(tensor=bv_d[:].tensor, offset=0,
                                   ap=[[0, 128], [1, DIM]]))
        bp_bc = const.tile([128, DIM], F32)
        nc.sync.dma_start(
            out=bp_bc, in_=bass.AP(tensor=bp_d[:].tensor, offset=0,
                                   ap=[[0, 128], [1, DIM]]))
        # host-gathered exp(rpe bias): eb2[p, h, cq] = exp_tab[idx[cq*128+p], h]
        eb2 = const.tile([128, H, 2 * N], F16)
        nc.sync.dma_start(out=eb2, in_=ebh_d[:])

        for rep in range(n_rep):
            # ------------- main loop over 4-window groups -------------
            for g in range(ngrp):
                tok0 = g * GW * N
                nc.gpsimd.dma_start(
                    out=x16_d[tok0:tok0 + GW * N, :],
                    in_=x_d[:].rearrange(
                        "w n c -> (w n) c")[tok0:tok0 + GW * N, :],
                )
                xt = xt_pool.tile([128, 4, GW * N], F16, tag="xt")
                for ci in range(4):
                    nc.sync.dma_start_transpose(
                        out=xt[:, ci, :],
                        in_=x16_d[tok0:tok0 + GW * N, ci * 128:(ci + 1) * 128],
                    )

                # Q^T / K^T  [o-chunk 128, tok] fp16
                qk_sb = qk_pool.tile([128, 8, GW * N], F16, tag="qk")
                for oc in range(8):
                    for half in range(2):
                        mm_ps = ps_a.tile([128, 512], F32, tag="ps_a")
                        for ci in range(4):
                            nc.tensor.matmul(
                                mm_ps[:, 0:392],
                                lhsT=wqk_sb[:, ci, oc * 128:(oc + 1) * 128],
                                rhs=xt[:, ci, half * 392:(half + 1) * 392],
                                start=(ci == 0), stop=(ci == 3),
                            )
                        nc.any.tensor_scalar_add(
                            out=qk_sb[:, oc, half * 392:(half + 1) * 392],
                            in0=mm_ps[:, 0:392],
                            scalar1=bqk_sb[:, oc:oc + 1],
                        )

                # V natural  [tok-chunk 98, h, 33] fp16 (col 32 = ones)
                v_sb = v_pool.tile([NKC, GW, 2, H, 33], F16, tag="v")
                nc.vector.memset(v_sb[:, :, :, :, 32:33], 1.0)
                for wi in range(GW):
                    for tcn in range(2):
                        vv_ps = ps_a.tile([128, 512], F32, tag="ps_a")
                        for ci in range(4):
                            nc.tensor.matmul(
                                vv_ps[0:NKC, :],
                                lhsT=xt[:, ci, wi * N + tcn * NKC:
                                        wi * N + (tcn + 1) * NKC],
                                rhs=wv_sb[:, ci, :],
                                start=(ci == 0), stop=(ci == 3),
                            )
                        nc.vector.tensor_add(
                            out=v_sb[:, wi, tcn, :, 0:32],
                            in0=vv_ps[0:NKC, :].rearrange(
                                "p (h d) -> p h d", h=H),
                            in1=bv_bc[0:NKC, :].rearrange(
                                "p (h d) -> p h d", h=H),
                        )

                # ---------------- attention per window ----------------
                for wi in range(GW):
                    w_abs = g * GW + wi
                    if stage < 2:
                        for qc in range(2):
                            y_sb = y_pool.tile([NKC, DIM], F32, tag="y")
                            nc.vector.memset(y_sb[:], 0.0)
                            nc.sync.dma_start(
                                out=out_d[w_abs, qc * NKC:(qc + 1) * NKC, :],
                                in_=y_sb[:])
                        continue
                    p_tiles = []
                    for hg in range(4):
                        p_raw = praw_pool.tile([NKC, 2, 4, N], F16,
                                               tag="praw")
                        p_sb = p_pool.tile([NKC, 2, 4, N], F16, tag="p")
                        p_tiles.append(p_sb)
                        s_ps = ps_s.tile([128, 4, 512], F32, tag="s")
                        for c in range(2):
                            for i in range(4):        # head = 4*hg + i
                                nc.tensor.matmul(
                                    s_ps[0:NKC, i, c * 196:c * 196 + 196],
                                    lhsT=qk_sb[32 * i:32 * (i + 1), 4 + hg,
                                               wi * N + c * NKC:
                                               wi * N + (c + 1) * NKC],
                                    rhs=qk_sb[32 * i:32 * (i + 1), hg,
                                              wi * N:(wi + 1) * N],
                                    start=True, stop=True,
                                    tile_position=(32 * i, 0),
                                )
                            nc.scalar.activation(
                                out=p_raw[:, c, :, :],
                                in_=s_ps[0:NKC, :, c * 196:c * 196 + 196],
                                func=mybir.ActivationFunctionType.Exp,
                                bias=mb_sb[:, 2 * w_abs + c:
                                           2 * w_abs + c + 1],
                                scale=1.0,
                            )
                            eb_eng = nc.gpsimd if hg % 2 else nc.vector
                            eb_eng.tensor_mul(
                                out=p_sb[:, c, :, :],
                                in0=p_raw[:, c, :, :],
                                in1=eb2[0:NKC, 4 * hg:4 * hg + 4,
                                        c * N:(c + 1) * N],
                            )

                    if stage < 3:
                        for qc in range(2):
                            y_sb = y_pool.tile([NKC, DIM], F32, tag="y")
                            nc.vector.memset(y_sb[:], 0.0)
                            nc.sync.dma_start(
                                out=out_d[w_abs, qc * NKC:(qc + 1) * NKC, :],
                                in_=y_sb[:])
                        continue
                    # PV+Z flipped: og[q, (h8, 33)] per (qb, grp)
                    o_sb = o_pool.tile([NKC, 2, DIM], F16, tag="o")
                    for qb in range(2):
                        for grp in range(2):
                            og = ps_og.tile([NKC, 512], F32, tag="og")
                            for h8 in range(8):
                                h = grp * 8 + h8
                                for c in range(2):
                                    nc.tensor.matmul(
                                        og[0:NKC, 33 * h8:33 * h8 + 33],
                                        lhsT=p_tiles[h // 4][
                                            :, c, h % 4,
                                            qb * NKC:(qb + 1) * NKC],
                                        rhs=v_sb[:, wi, c, h, :],
                                        start=(c == 0), stop=(c == 1),
                                    )
                            if stage < 4:
                                y_sb = y_pool.tile([NKC, DIM], F32, tag="y")
                                nc.vector.tensor_copy(
                                    out=y_sb[:, 0:264], in_=og[0:NKC, 0:264])
                                nc.vector.memset(y_sb[:, 264:512], 0.0)
                                nc.sync.dma_start(
                                    out=out_d[w_abs,
                                              qb * NKC:(qb + 1) * NKC, :],
                                    in_=y_sb[:])
                                continue
                            ogv = og[0:NKC, 0:264].rearrange(
                                "p (h d) -> p h d", d=33)
                            rz = rz_pool.tile([NKC, 8], F32, tag="rz")
                            nc.vector.reciprocal(out=rz[:], in_=ogv[:, :, 32])
                            if no_bcast:
                                for h8 in range(8):
                                    nc.vector.tensor_scalar_mul(
                                        out=o_sb[:, qb,
                                                 grp * 256 + 32 * h8:
                                                 grp * 256 + 32 * h8 + 32],
                                        in0=ogv[:, h8, 0:32],
                                        scalar1=rz[:, h8:h8 + 1],
                                    )
                            else:
                                nc.vector.tensor_mul(
                                    out=o_sb[:, qb, grp * 256:grp * 256 + 256
                                             ].rearrange(
                                                 "p (h d) -> p h d", h=8),
                                    in0=ogv[:, :, 0:32],
                                    in1=rz[:].broadcast_to((NKC, 8, 32)),
                                )

                    if stage < 4:
                        continue
                    if stage < 5:
                        for qc in range(2):
                            y_sb = y_pool.tile([NKC, DIM], F32, tag="y")
                            nc.vector.tensor_copy(
                                out=y_sb[:], in_=o_sb[:, qc, :])
                            nc.sync.dma_start(
                                out=out_d[w_abs, qc * NKC:(qc + 1) * NKC, :],
                                in_=y_sb[:])
                        continue
                    # transpose O -> O^T and proj
                    ot_sb = ot_pool.tile([128, 4, 2, NKC], F16, tag="ot")
                    if tr_f32:
                        o32_sb = o_pool.tile([NKC, 2, DIM], F32, tag="o32")
                        nc.scalar.activation(
                            out=o32_sb[:], in_=o_sb[:],
                            func=mybir.ActivationFunctionType.Copy)
                    for cc in range(4):
                        for qb in range(2):
                            if tr_f32:
                                ot_ps = ps_a.tile([128, 512], F32,
                                                  tag="ps_a")
                                nc.tensor.transpose(
                                    out=ot_ps[:, 0:NKC],
                                    in_=o32_sb[:, qb,
                                               cc * 128:(cc + 1) * 128],
                                    identity=ident32_sb[0:NKC, 0:NKC],
                                )
                            else:
                                ot_ps = ps_a.tile([128, 1024], F16,
                                                  tag="ps_a")
                                nc.tensor.transpose(
                                    out=ot_ps[:, 0:NKC],
                                    in_=o_sb[:, qb, cc * 128:(cc + 1) * 128],
                                    identity=ident_sb[0:NKC, 0:NKC],
                                )
                            nc.vector.tensor_copy(
                                out=ot_sb[:, cc, qb, :],
                                in_=ot_ps[:, 0:NKC],
                            )
                    if stage < 6:
                        for qc in range(2):
                            y_sb = y_pool.tile([NKC, DIM], F32, tag="y")
                            nc.vector.tensor_copy(
                                out=y_sb[:],
                                in_=ot_sb[:, :, qc, :].rearrange(
                                    "p a b -> p (a b)")[0:NKC, 0:DIM])
                            nc.sync.dma_start(
                                out=out_d[w_abs, qc * NKC:(qc + 1) * NKC, :],
                                in_=y_sb[:])
                        continue
                    for qc in range(2):
                        y_ps = ps_a.tile([128, 512], F32, tag="ps_a")
                        for cc in range(4):
                            nc.tensor.matmul(
                                y_ps[0:NKC, :],
                                lhsT=ot_sb[:, cc, qc, :],
                                rhs=wp_sb[:, cc, :],
                                start=(cc == 0), stop=(cc == 3),
                            )
                        y_sb = y_pool.tile([NKC, DIM], F32, tag="y")
                        nc.vector.tensor_add(
                            out=y_sb[:], in0=y_ps[0:NKC, :],
                            in1=bp_bc[0:NKC, :])
                        nc.sync.dma_start(
                            out=out_d[w_abs, qc * NKC:(qc + 1) * NKC, :],
                            in_=y_sb[:],
                        )
    nc.compile()
    return nc


def _host_prep(x, rpe_index, mask, qkv_w, qkv_b, proj_w, proj_b, rpe_table,
               n_w=W, n_cores=NCORES, exp_tab=True):
    """Shard + layout/dtype prep (numpy only). Returns per-core input maps."""
    x = np.asarray(x, dtype=np.float32)
    rpe_index = np.asarray(rpe_index).astype(np.int64)
    mask = np.asarray(mask).astype(np.int32)
    qkv_w = np.asarray(qkv_w, dtype=np.float32)
    qkv_b = np.asarray(qkv_b, dtype=np.float32)
    proj_w = np.asarray(proj_w, dtype=np.float32)
    proj_b = np.asarray(proj_b, dtype=np.float32)
    rpe_table = np.asarray(rpe_table, dtype=np.float32)

    scale = HD ** -0.5
    wq = qkv_w[0:DIM] * scale
    wk = qkv_w[DIM:2 * DIM]
    wv = qkv_w[2 * DIM:3 * DIM]
    wqk_t = np.concatenate([wq, wk], axis=0).T.astype(np.float16)  # [c, 1024]
    wv_t = wv.T.astype(np.float16)                                 # [c, 512]
    wp_t = proj_w.T.astype(np.float16)                             # [c, 512]
    wqk_t = np.ascontiguousarray(wqk_t.reshape(4, 128, 2 * DIM))
    wv_t = np.ascontiguousarray(wv_t.reshape(4, 128, DIM))
    wp_t = np.ascontiguousarray(wp_t.reshape(4, 128, DIM))

    bqk = np.concatenate([qkv_b[0:DIM] * scale, qkv_b[DIM:2 * DIM]])
    bqk_pp = np.ascontiguousarray(
        bqk.reshape(8, 128).T.astype(np.float32))                  # [128, 8]
    bv = qkv_b[2 * DIM:3 * DIM].astype(np.float32)

    tab = np.zeros((RPE, 128), dtype=np.float16)
    tab_vals = np.exp(rpe_table) if exp_tab else rpe_table
    tab[:, 0:H] = tab_vals.astype(np.float16)
    tab16 = np.ascontiguousarray(tab[:, 0:H])

    # gather index stream: position j = cq*128 + p ; cq = c*196+q ; k = 98c+p
    cq = np.arange(2 * N)
    c = cq // N
    q = cq % N
    p = np.arange(128)
    k = (NKC * c)[:, None] + p[None, :]                            # [392, 128]
    valid = p[None, :] < NKC
    j_idx = np.where(valid, rpe_index[q[:, None] * N + np.minimum(k, N - 1)], 0)
    j_idx = j_idx.reshape(-1).astype(np.int16)
    n_per = _GQ * 128
    idx_w = np.zeros((16, (n_per // 16) * _NGATHER), dtype=np.int16)
    for gch in range(_NGATHER):
        blk = j_idx[gch * n_per:(gch + 1) * n_per].reshape(n_per // 16, 16).T
        idx_w[:, gch * (n_per // 16):(gch + 1) * (n_per // 16)] = blk
    idx_w = np.ascontiguousarray(np.tile(idx_w, (8, 1)))           # [128, .]

    # host-gathered bias table: ebh[p, h, cq] = tab[j_idx[cq, p], h]
    j2 = j_idx.reshape(2 * N, 128)                                 # [cq, p]
    ebh = np.ascontiguousarray(
        tab[j2.astype(np.int64), 0:H].transpose(1, 2, 0))          # [p, h, cq]

    ident = np.eye(128, dtype=np.float16)

    in_maps = []
    for core in range(n_cores):
        xs = x[core * n_w:(core + 1) * n_w]
        ms = mask[core * n_w:(core + 1) * n_w]
        mbv = np.where(ms.astype(bool), EXP_SHIFT, MASK_NEG).astype(np.float32)
        mb = np.zeros((NKC, n_w * 2), dtype=np.float32)
        for wi in range(n_w):
            for cc in range(2):
                mb[:, 2 * wi + cc] = mbv[wi, cc * NKC:(cc + 1) * NKC]
        in_maps.append({
            "x": np.ascontiguousarray(xs),
            "wqk": wqk_t, "wv": wv_t, "wp": wp_t,
            "bqk": bqk_pp, "bv": bv, "bp": proj_b.astype(np.float32),
            "tab": tab, "ebh": ebh, "idx": idx_w,
            "mb": np.ascontiguousarray(mb),
            "ident": ident,
        })
    return in_maps


_NC_CACHE = {}
_BUILDER = _build_v3


def kernel(x, rpe_index, mask, qkv_w, qkv_b, proj_w, proj_b, rpe_table,
           _trace=False):
    from concourse.bass_utils import run_bass_kernel_spmd
    in_maps = _host_prep(x, rpe_index, mask, qkv_w, qkv_b, proj_w, proj_b,
                         rpe_table)
    if "nc" not in _NC_CACHE:
        _NC_CACHE["nc"] = _BUILDER()
    nc = _NC_CACHE["nc"]
    try:
        res = run_bass_kernel_spmd(nc, in_maps, core_ids=list(range(NCORES)),
                                   trace=_trace)
    except ModuleNotFoundError:
        # axon NTFF profiling hook unavailable in this container
        res = run_bass_kernel_spmd(nc, in_maps, core_ids=list(range(NCORES)),
                                   trace=False)
    kernel.last_results = res
    out = np.concatenate([r["out"] for r in res.results], axis=0)
    return out.reshape(B, N, DIM).astype(np.float32)

